# revision 24
# baseline (speedup 1.0000x reference)
"""Multi-head attention (B=4, S=2048, D=1024, H=16, Hd=64) on 8 trn2 cores.

Sharding: core c = (batch b = c // 2, head-group hg = c % 2). Each core
computes attention for 8 heads of one batch and the corresponding slice of
the output projection; host sums the two partial outputs per batch.

Per-core layout (all matmuls bf16 with fp32 PSUM accumulation):
  xt   = x[b].T                    [D=1024, S=2048]  (lhsT/rhs K-major)
  qT/kT = (Wslice.T @ .. )         [512, 2048]  d-major, 4 pair-tiles of 128
  v    = x @ Wv_slice              [2048, 512]  s-major
  per head: scoresT[k,q] tile = kT.T-block @ qT  -> exp (ScalarE, scale=1/8)
            outT[d,q] += v-block.T @ expT ; denom[q] += ones.T @ expT
  normalize: outT *= broadcast(1/denom) via K=1 ones-matmul
  y = outT.T-blocks @ Wo_slice + bo   [2048, 1024] fp32 partial
"""

import numpy as np
import ml_dtypes

S = 2048
D = 1024
HG_D = 512          # head dims per core (8 heads x 64)
NH = 8              # heads per core
KT = S // 128       # 16 k-tiles
DT = D // 128       # 8 contraction tiles for QKV
ST = S // 128       # 16 s-tiles
OT = HG_D // 128    # 4 contraction tiles for O-proj / pair tiles
N_CORES = 8

BF16 = ml_dtypes.bfloat16

_CACHED_NC = {}


def _build_nc(with_bv=True, with_bo=True):
    import concourse.bass as bass  # noqa: F401
    import concourse.mybir as mybir
    import concourse.tile as tile
    from concourse import bacc

    f32 = mybir.dt.float32
    bf16 = mybir.dt.bfloat16
    fp16 = mybir.dt.float16
    Exp = mybir.ActivationFunctionType.Exp

    nc = bacc.Bacc("TRN2", target_bir_lowering=False, debug=False,
                   num_devices=N_CORES)

    xt_d = nc.dram_tensor("xt", [D, S], bf16, kind="ExternalInput")
    wq_d = nc.dram_tensor("wq", [D, HG_D], bf16, kind="ExternalInput")
    wk_d = nc.dram_tensor("wk", [D, HG_D], bf16, kind="ExternalInput")
    wv_d = nc.dram_tensor("wv", [D, HG_D], bf16, kind="ExternalInput")
    wo_d = nc.dram_tensor("wo", [HG_D, D], bf16, kind="ExternalInput")
    bq_d = nc.dram_tensor("bqt", [128, OT], f32, kind="ExternalInput")
    bk_d = nc.dram_tensor("bkt", [128, OT], f32, kind="ExternalInput")
    bv_d = nc.dram_tensor("bvr", [1, HG_D], bf16, kind="ExternalInput")
    bo_d = nc.dram_tensor("bor", [1, D], bf16, kind="ExternalInput")
    y_d = nc.dram_tensor("y", [S, D], f32, kind="ExternalOutput")

    with tile.TileContext(nc) as tc:
        with (
            tc.tile_pool(name="cpool", bufs=1) as cpool,
            tc.tile_pool(name="wpool", bufs=2) as wpool,
            tc.tile_pool(name="pspool", bufs=2, space="PSUM") as pspool,
            tc.tile_pool(name="popool", bufs=2, space="PSUM") as popool,
        ):
            # ---- persistent SBUF tiles ----
            xt_sb = cpool.tile([128, DT, S], bf16, name="xt_sb")
            wq_sb = cpool.tile([128, DT, HG_D], bf16, name="wq_sb")
            wk_sb = cpool.tile([128, DT, HG_D], bf16, name="wk_sb")
            wv_sb = cpool.tile([128, DT, HG_D], bf16, name="wv_sb")
            wo_sb = cpool.tile([128, OT, D], bf16, name="wo_sb")
            bq_sb = cpool.tile([128, OT], f32, name="bq_sb")
            bk_sb = cpool.tile([128, OT], f32, name="bk_sb")
            bvr_sb = cpool.tile([1, HG_D], bf16, name="bvr_sb")
            bor_sb = cpool.tile([1, D], bf16, name="bor_sb")
            ones_t = cpool.tile([128, 128], bf16, name="ones_t")
            qT_sb = cpool.tile([128, OT, S], bf16, name="qT_sb")
            kT_sb = cpool.tile([128, OT, S], bf16, name="kT_sb")
            # v with a trailing ones column per head: attnv lhsT [128, 65]
            # whose 65th output row accumulates the softmax denominator.
            v_sb = cpool.tile([128, ST, NH, 65], bf16, name="v_sb")
            aoT_sb = cpool.tile([128, OT, S], bf16, name="aoT_sb")

            # ---- loads ----
            for k in range(DT):
                nc.sync.dma_start(out=xt_sb[:, k, :], in_=xt_d[k * 128:(k + 1) * 128, :])
                nc.sync.dma_start(out=wq_sb[:, k, :], in_=wq_d[k * 128:(k + 1) * 128, :])
                nc.sync.dma_start(out=wk_sb[:, k, :], in_=wk_d[k * 128:(k + 1) * 128, :])
                nc.sync.dma_start(out=wv_sb[:, k, :], in_=wv_d[k * 128:(k + 1) * 128, :])
            for k in range(OT):
                nc.sync.dma_start(out=wo_sb[:, k, :], in_=wo_d[k * 128:(k + 1) * 128, :])
            nc.sync.dma_start(out=bq_sb[:], in_=bq_d[:])
            nc.sync.dma_start(out=bk_sb[:], in_=bk_d[:])
            nc.sync.dma_start(out=bvr_sb[:], in_=bv_d[:])
            nc.sync.dma_start(out=bor_sb[:], in_=bo_d[:])
            nc.gpsimd.memset(ones_t[:], 1.0)

            # ---- projections: qT/kT [d-major], bias via per-partition scalar ----
            def emit_qk_group(w_sb, b_sb, out_sb, p, jc):
                pq = pspool.tile([128, 512], f32, tag="pp", name="pq")
                for k in range(DT):
                    nc.tensor.matmul(
                        pq[:],
                        w_sb[:, k, p * 128:(p + 1) * 128],
                        xt_sb[:, k, jc * 512:(jc + 1) * 512],
                        start=(k == 0), stop=(k == DT - 1),
                    )
                nc.vector.tensor_scalar_add(
                    out_sb[:, p, jc * 512:(jc + 1) * 512], pq[:], b_sb[:, p:p + 1])

            for w_sb, b_sb, out_sb in ((wq_sb, bq_sb, qT_sb), (wk_sb, bk_sb, kT_sb)):
                for jc in range(4):
                    emit_qk_group(w_sb, b_sb, out_sb, 0, jc)

            # ---- v projection [s-major], bias via K=1 ones matmul ----
            nc.vector.memset(v_sb[:], 1.0)
            for st in range(ST):
                pv = pspool.tile([128, 512], f32, tag="pp", name="pv")
                for k in range(DT):
                    nc.tensor.matmul(
                        pv[:],
                        xt_sb[:, k, st * 128:(st + 1) * 128],
                        wv_sb[:, k, :],
                        start=(k == 0), stop=(not with_bv and k == DT - 1),
                    )
                if with_bv:
                    nc.tensor.matmul(pv[:], ones_t[0:1, 0:128], bvr_sb[0:1, :],
                                     start=False, stop=True)
                nc.vector.tensor_copy(
                    v_sb[:, st, :, 0:64],
                    pv.rearrange("p (h c) -> p h c", c=64))

            # ---- attention, one head PAIR per chunk ----
            # Both heads of a pair are processed together so consecutive
            # scores matmuls alternate PE row halves (rows 0-63 / 64-127),
            # which lets the hardware overlap them (~2x scores throughput).
            # Normalization (broadcast of 1/denom + scale) is deferred by one
            # chunk so the PE never waits on the slow DVE reciprocal.
            pending = []

            def emit_oproj_group(st):
                yt = wpool.tile([128, D], f32, tag="y", name="yt")
                for l in range(2):
                    py = pspool.tile([128, 512], f32, tag="pp", name="py")
                    for kt in range(OT):
                        nc.tensor.matmul(
                            py[:],
                            aoT_sb[:, kt, st * 128:(st + 1) * 128],
                            wo_sb[:, kt, l * 512:(l + 1) * 512],
                            start=(kt == 0), stop=(not with_bo and kt == OT - 1),
                        )
                    if with_bo:
                        nc.tensor.matmul(py[:], ones_t[0:1, 0:128],
                                         bor_sb[0:1, l * 512:(l + 1) * 512],
                                         start=False, stop=True)
                    nc.vector.tensor_copy(yt[:, l * 512:(l + 1) * 512], py[:])
                nc.sync.dma_start(out=y_d[st * 128:(st + 1) * 128, :], in_=yt[:])

            def flush_normalize():
                off2, p2, jb2, rb2, row = pending.pop(0)
                bt = pspool.tile([128, 512], f32, tag="ps", name="bt")
                nc.tensor.matmul(
                    bt[off2:off2 + 64, :],
                    ones_t[row:row + 1, 0:64],
                    rb2[row:row + 1, :],
                    start=True, stop=True,
                )
                nc.vector.tensor_mul(
                    aoT_sb[off2:off2 + 64, p2, jb2],
                    aoT_sb[off2:off2 + 64, p2, jb2],
                    bt[off2:off2 + 64, :])

            oproj_done = set()
            for p in range(OT):
                deferred = []
                if p + 1 < OT:
                    for w_sb, b_sb, out_sb in ((wq_sb, bq_sb, qT_sb),
                                               (wk_sb, bk_sb, kT_sb)):
                        for jc in range(4):
                            deferred.append(
                                (emit_qk_group, (w_sb, b_sb, out_sb, p + 1, jc)))
                for c in range(4):
                    jb = slice(c * 512, (c + 1) * 512)
                    while len(pending) > 2:
                        flush_normalize()
                    otA = popool.tile([65, 512], f32, tag="po", name="otA")
                    otB = popool.tile([65, 512], f32, tag="po", name="otB")
                    if p == OT - 1 and c >= 2:
                        # s-rows of chunks <= c-2 are normalized on all pairs;
                        # start the output projection early for those tiles.
                        limit = 4 * (c - 1)
                        for st in range(limit):
                            if st not in oproj_done and len(oproj_done) < 2 * (c - 1):
                                deferred.append((emit_oproj_group, (st,)))
                                oproj_done.add(st)
                    for i in range(KT):
                        if i % 8 == 4 and deferred:
                            fn, args = deferred.pop(0)
                            fn(*args)
                        # combined scores psum: head A in cols 0-511 (rows
                        # 0-63 of the PE array), head B in cols 512-1023
                        # (rows 64-127) -- consecutive matmuls alternate PE
                        # row halves so the array overlaps them.
                        stt = pspool.tile([128, 1024], f32, tag="ps",
                                          name="stt")
                        for off in (0, 64):
                            nc.tensor.matmul(
                                stt[:, off * 8:off * 8 + 512],
                                kT_sb[off:off + 64, p, i * 128:(i + 1) * 128],
                                qT_sb[off:off + 64, p, jb],
                                start=True, stop=True,
                            )
                        et = wpool.tile([128, 1024], bf16, tag="exp", bufs=3,
                                        name="et")
                        nc.scalar.activation(et[:], stt[:], Exp, scale=0.125)
                        for ot, hh in ((otA, 0), (otB, 1)):
                            nc.tensor.matmul(
                                ot[:],
                                v_sb[:, i, 2 * p + hh, :],
                                et[:, hh * 512:(hh + 1) * 512],
                                start=(i == 0), stop=(i == KT - 1),
                            )
                    # Drain attn rows (cross-partition for the odd head) first
                    # so the PSUM tiles free fast, then gather the denominator
                    # rows to partitions 64 (A) / 32 (B) and batch-reciprocal.
                    nc.vector.tensor_copy(aoT_sb[0:64, p, jb], otA[0:64, :])
                    nc.vector.tensor_copy(aoT_sb[64:128, p, jb], otB[0:64, :])
                    den = wpool.tile([97, 512], f32, tag="den", bufs=2,
                                     name="den")
                    nc.vector.memset(den[64:97, :], 1.0)
                    nc.vector.tensor_copy(den[64:65, :], otA[64:65, :])
                    nc.vector.tensor_copy(den[96:97, :], otB[64:65, :])
                    rf = wpool.tile([97, 512], f32, tag="rf", name="rf")
                    rb = wpool.tile([65, 512], bf16, tag="rb", bufs=3,
                                    name="rb")
                    nc.vector.reciprocal(rf[64:97, :], den[64:97, :])
                    nc.vector.tensor_copy(rb[64:65, :], rf[64:65, :])
                    nc.vector.tensor_copy(rb[32:33, :], rf[96:97, :])
                    pending.append((0, p, jb, rb, 64))
                    pending.append((64, p, jb, rb, 32))
            while pending:
                flush_normalize()

            # ---- remaining output projection ----
            for st in range(ST):
                if st not in oproj_done:
                    emit_oproj_group(st)

    nc.compile()
    return nc


def get_nc(with_bv=True, with_bo=True):
    key = (with_bv, with_bo)
    if key not in _CACHED_NC:
        _CACHED_NC[key] = _build_nc(*key)
    return _CACHED_NC[key]


def make_in_maps(x, Wq, bq, Wk, bk, Wv, bv, Wo, bo):
    x = np.asarray(x, dtype=np.float32)
    in_maps = []
    for c in range(N_CORES):
        b, hg = c // 2, c % 2
        sl = slice(hg * HG_D, (hg + 1) * HG_D)
        in_maps.append({
            "xt": np.ascontiguousarray(np.asarray(x[b]).T).astype(BF16),
            "wq": np.ascontiguousarray(np.asarray(Wq)[:, sl]).astype(BF16),
            "wk": np.ascontiguousarray(np.asarray(Wk)[:, sl]).astype(BF16),
            "wv": np.ascontiguousarray(np.asarray(Wv)[:, sl]).astype(BF16),
            "wo": np.ascontiguousarray(np.asarray(Wo)[sl, :]).astype(BF16),
            "bqt": np.ascontiguousarray(
                np.asarray(bq, np.float32)[sl].reshape(OT, 128).T),
            "bkt": np.ascontiguousarray(
                np.asarray(bk, np.float32)[sl].reshape(OT, 128).T),
            "bvr": np.asarray(bv, np.float32)[sl].reshape(1, HG_D).astype(BF16),
            "bor": (np.asarray(bo, np.float32) if hg == 0
                    else np.zeros(D, np.float32)).reshape(1, D).astype(BF16),
        })
    return in_maps


def run_cores(in_maps, trace=False, with_bv=True, with_bo=True):
    try:
        import ntff_shim
        ntff_shim.install()
    except Exception:
        pass
    from concourse.bass_utils import run_bass_kernel_spmd

    nc = get_nc(with_bv, with_bo)
    return run_bass_kernel_spmd(nc, in_maps, list(range(N_CORES)), trace=trace)


def combine(results):
    y = np.empty((4, S, D), np.float32)
    for b in range(4):
        y[b] = results[2 * b]["y"] + results[2 * b + 1]["y"]
    return y


def kernel(x, Wq, bq, Wk, bk, Wv, bv, Wo, bo):
    in_maps = make_in_maps(x, Wq, bq, Wk, bk, Wv, bv, Wo, bo)
    with_bv = bool(np.any(np.asarray(bv)))
    with_bo = bool(np.any(np.asarray(bo)))
    res = run_cores(in_maps, trace=False, with_bv=with_bv, with_bo=with_bo)
    return combine(res.results)


# revision 25
# speedup vs baseline: 1.0711x; 1.0711x over previous
"""Multi-head attention (B=4, S=2048, D=1024, H=16, Hd=64) on 8 trn2 cores.

Sharding: core c = (batch b = c // 2, head-group hg = c % 2). Each core
computes attention for 8 heads of one batch and the corresponding slice of
the output projection; host sums the two partial outputs per batch.

Per-core layout (all matmuls bf16 with fp32 PSUM accumulation):
  xt   = x[b].T                    [D=1024, S=2048]  (lhsT/rhs K-major)
  qT/kT = (Wslice.T @ .. )         [512, 2048]  d-major, 4 pair-tiles of 128
  v    = x @ Wv_slice              [2048, 512]  s-major
  per head: scoresT[k,q] tile = kT.T-block @ qT  -> exp (ScalarE, scale=1/8)
            outT[d,q] += v-block.T @ expT ; denom[q] += ones.T @ expT
  normalize: outT *= broadcast(1/denom) via K=1 ones-matmul
  y = outT.T-blocks @ Wo_slice + bo   [2048, 1024] fp32 partial
"""

import numpy as np
import ml_dtypes

S = 2048
D = 1024
HG_D = 512          # head dims per core (8 heads x 64)
NH = 8              # heads per core
KT = S // 128       # 16 k-tiles
DT = D // 128       # 8 contraction tiles for QKV
ST = S // 128       # 16 s-tiles
OT = HG_D // 128    # 4 contraction tiles for O-proj / pair tiles
N_CORES = 8

BF16 = ml_dtypes.bfloat16

_CACHED_NC = {}


def _build_nc(with_bv=True, with_bo=True):
    import concourse.bass as bass  # noqa: F401
    import concourse.mybir as mybir
    import concourse.tile as tile
    from concourse import bacc

    f32 = mybir.dt.float32
    bf16 = mybir.dt.bfloat16
    fp16 = mybir.dt.float16
    Exp = mybir.ActivationFunctionType.Exp

    nc = bacc.Bacc("TRN2", target_bir_lowering=False, debug=False,
                   num_devices=N_CORES)

    xt_d = nc.dram_tensor("xt", [D, S], bf16, kind="ExternalInput")
    wq_d = nc.dram_tensor("wq", [D, HG_D], bf16, kind="ExternalInput")
    wk_d = nc.dram_tensor("wk", [D, HG_D], bf16, kind="ExternalInput")
    wv_d = nc.dram_tensor("wv", [D, HG_D], bf16, kind="ExternalInput")
    wo_d = nc.dram_tensor("wo", [HG_D, D], bf16, kind="ExternalInput")
    bq_d = nc.dram_tensor("bqt", [128, OT], f32, kind="ExternalInput")
    bk_d = nc.dram_tensor("bkt", [128, OT], f32, kind="ExternalInput")
    bv_d = nc.dram_tensor("bvr", [1, HG_D], bf16, kind="ExternalInput")
    bo_d = nc.dram_tensor("bor", [1, D], bf16, kind="ExternalInput")
    y_d = nc.dram_tensor("y", [S, D], f32, kind="ExternalOutput")

    with tile.TileContext(nc) as tc:
        with (
            tc.tile_pool(name="cpool", bufs=1) as cpool,
            tc.tile_pool(name="wpool", bufs=2) as wpool,
            tc.tile_pool(name="pspool", bufs=3, space="PSUM") as pspool,
            tc.tile_pool(name="popool", bufs=2, space="PSUM") as popool,
        ):
            # ---- persistent SBUF tiles ----
            xt_sb = cpool.tile([128, DT, S], bf16, name="xt_sb")
            wq_sb = cpool.tile([128, DT, HG_D], bf16, name="wq_sb")
            wk_sb = cpool.tile([128, DT, HG_D], bf16, name="wk_sb")
            wv_sb = cpool.tile([128, DT, HG_D], bf16, name="wv_sb")
            wo_sb = cpool.tile([128, OT, D], bf16, name="wo_sb")
            bq_sb = cpool.tile([128, OT], f32, name="bq_sb")
            bk_sb = cpool.tile([128, OT], f32, name="bk_sb")
            bvr_sb = cpool.tile([1, HG_D], bf16, name="bvr_sb")
            bor_sb = cpool.tile([1, D], bf16, name="bor_sb")
            ones_t = cpool.tile([128, 128], bf16, name="ones_t")
            qT_sb = cpool.tile([128, OT, S], bf16, name="qT_sb")
            kT_sb = cpool.tile([128, OT, S], bf16, name="kT_sb")
            # v with a trailing ones column per head: attnv lhsT [128, 65]
            # whose 65th output row accumulates the softmax denominator.
            v_sb = cpool.tile([128, ST, NH, 65], bf16, name="v_sb")
            aoT_sb = cpool.tile([128, OT, S], bf16, name="aoT_sb")

            # ---- loads ----
            for k in range(DT):
                nc.sync.dma_start(out=xt_sb[:, k, :], in_=xt_d[k * 128:(k + 1) * 128, :])
                nc.sync.dma_start(out=wq_sb[:, k, :], in_=wq_d[k * 128:(k + 1) * 128, :])
                nc.sync.dma_start(out=wk_sb[:, k, :], in_=wk_d[k * 128:(k + 1) * 128, :])
                nc.sync.dma_start(out=wv_sb[:, k, :], in_=wv_d[k * 128:(k + 1) * 128, :])
            for k in range(OT):
                nc.sync.dma_start(out=wo_sb[:, k, :], in_=wo_d[k * 128:(k + 1) * 128, :])
            nc.sync.dma_start(out=bq_sb[:], in_=bq_d[:])
            nc.sync.dma_start(out=bk_sb[:], in_=bk_d[:])
            nc.sync.dma_start(out=bvr_sb[:], in_=bv_d[:])
            nc.sync.dma_start(out=bor_sb[:], in_=bo_d[:])
            nc.gpsimd.memset(ones_t[:], 1.0)

            # ---- projections: qT/kT [d-major], bias via per-partition scalar ----
            def emit_qk_group(w_sb, b_sb, out_sb, p, jc):
                pq = pspool.tile([128, 512], f32, tag="ps", name="pq")
                for k in range(DT):
                    nc.tensor.matmul(
                        pq[:],
                        w_sb[:, k, p * 128:(p + 1) * 128],
                        xt_sb[:, k, jc * 512:(jc + 1) * 512],
                        start=(k == 0), stop=(k == DT - 1),
                    )
                nc.vector.tensor_scalar_add(
                    out_sb[:, p, jc * 512:(jc + 1) * 512], pq[:], b_sb[:, p:p + 1])

            for w_sb, b_sb, out_sb in ((wq_sb, bq_sb, qT_sb), (wk_sb, bk_sb, kT_sb)):
                for p in range(OT):
                    for jc in range(4):
                        emit_qk_group(w_sb, b_sb, out_sb, p, jc)

            # ---- v projection [s-major], bias via K=1 ones matmul ----
            nc.vector.memset(v_sb[:], 1.0)
            for st in range(ST):
                pv = pspool.tile([128, 512], f32, tag="ps", name="pv")
                for k in range(DT):
                    nc.tensor.matmul(
                        pv[:],
                        xt_sb[:, k, st * 128:(st + 1) * 128],
                        wv_sb[:, k, :],
                        start=(k == 0), stop=(not with_bv and k == DT - 1),
                    )
                if with_bv:
                    nc.tensor.matmul(pv[:], ones_t[0:1, 0:128], bvr_sb[0:1, :],
                                     start=False, stop=True)
                nc.vector.tensor_copy(
                    v_sb[:, st, :, 0:64],
                    pv.rearrange("p (h c) -> p h c", c=64))

            # ---- attention, one head PAIR per chunk ----
            # Both heads of a pair are processed together so consecutive
            # scores matmuls alternate PE row halves (rows 0-63 / 64-127),
            # which lets the hardware overlap them (~2x scores throughput).
            # Normalization (broadcast of 1/denom + scale) is deferred by one
            # chunk so the PE never waits on the slow DVE reciprocal.
            pending = []

            def emit_oproj_group(st):
                yt = wpool.tile([128, D], f32, tag="y", name="yt")
                for l in range(2):
                    py = pspool.tile([128, 512], f32, tag="ps", name="py")
                    for kt in range(OT):
                        nc.tensor.matmul(
                            py[:],
                            aoT_sb[:, kt, st * 128:(st + 1) * 128],
                            wo_sb[:, kt, l * 512:(l + 1) * 512],
                            start=(kt == 0), stop=(not with_bo and kt == OT - 1),
                        )
                    if with_bo:
                        nc.tensor.matmul(py[:], ones_t[0:1, 0:128],
                                         bor_sb[0:1, l * 512:(l + 1) * 512],
                                         start=False, stop=True)
                    nc.vector.tensor_copy(yt[:, l * 512:(l + 1) * 512], py[:])
                nc.sync.dma_start(out=y_d[st * 128:(st + 1) * 128, :], in_=yt[:])

            def flush_normalize():
                off2, p2, jb2, rb2, row = pending.pop(0)
                bt = pspool.tile([128, 512], f32, tag="ps", name="bt")
                nc.tensor.matmul(
                    bt[off2:off2 + 64, :],
                    ones_t[row:row + 1, 0:64],
                    rb2[row:row + 1, :],
                    start=True, stop=True,
                )
                nc.vector.tensor_mul(
                    aoT_sb[off2:off2 + 64, p2, jb2],
                    aoT_sb[off2:off2 + 64, p2, jb2],
                    bt[off2:off2 + 64, :])

            oproj_done = set()
            for p in range(OT):
                for c in range(4):
                    jb = slice(c * 512, (c + 1) * 512)
                    while len(pending) > 2:
                        flush_normalize()
                    otA = popool.tile([65, 512], f32, tag="po", name="otA")
                    otB = popool.tile([65, 512], f32, tag="po", name="otB")
                    for i in range(KT):
                        # combined scores psum: head A in cols 0-511 (rows
                        # 0-63 of the PE array), head B in cols 512-1023
                        # (rows 64-127) -- consecutive matmuls alternate PE
                        # row halves so the array overlaps them.
                        stt = pspool.tile([128, 1024], f32, tag="ps",
                                          name="stt")
                        for off in (0, 64):
                            nc.tensor.matmul(
                                stt[:, off * 8:off * 8 + 512],
                                kT_sb[off:off + 64, p, i * 128:(i + 1) * 128],
                                qT_sb[off:off + 64, p, jb],
                                start=True, stop=True,
                            )
                        et = wpool.tile([128, 1024], bf16, tag="exp", bufs=3,
                                        name="et")
                        nc.scalar.activation(et[:], stt[:], Exp, scale=0.125)
                        for ot, hh in ((otA, 0), (otB, 1)):
                            nc.tensor.matmul(
                                ot[:],
                                v_sb[:, i, 2 * p + hh, :],
                                et[:, hh * 512:(hh + 1) * 512],
                                start=(i == 0), stop=(i == KT - 1),
                            )
                    # Drain attn rows (cross-partition for the odd head) first
                    # so the PSUM tiles free fast, then gather the denominator
                    # rows to partitions 64 (A) / 32 (B) and batch-reciprocal.
                    nc.vector.tensor_copy(aoT_sb[0:64, p, jb], otA[0:64, :])
                    nc.vector.tensor_copy(aoT_sb[64:128, p, jb], otB[0:64, :])
                    den = wpool.tile([97, 512], f32, tag="den", bufs=2,
                                     name="den")
                    nc.vector.memset(den[64:97, :], 1.0)
                    nc.vector.tensor_copy(den[64:65, :], otA[64:65, :])
                    nc.vector.tensor_copy(den[96:97, :], otB[64:65, :])
                    rf = wpool.tile([97, 512], f32, tag="rf", name="rf")
                    rb = wpool.tile([65, 512], bf16, tag="rb", bufs=3,
                                    name="rb")
                    nc.vector.reciprocal(rf[64:97, :], den[64:97, :])
                    nc.vector.tensor_copy(rb[64:65, :], rf[64:65, :])
                    nc.vector.tensor_copy(rb[32:33, :], rf[96:97, :])
                    pending.append((0, p, jb, rb, 64))
                    pending.append((64, p, jb, rb, 32))
            while pending:
                flush_normalize()

            # ---- remaining output projection ----
            for st in range(ST):
                if st not in oproj_done:
                    emit_oproj_group(st)

    nc.compile()
    return nc


def get_nc(with_bv=True, with_bo=True):
    key = (with_bv, with_bo)
    if key not in _CACHED_NC:
        _CACHED_NC[key] = _build_nc(*key)
    return _CACHED_NC[key]


def make_in_maps(x, Wq, bq, Wk, bk, Wv, bv, Wo, bo):
    x = np.asarray(x, dtype=np.float32)
    in_maps = []
    for c in range(N_CORES):
        b, hg = c // 2, c % 2
        sl = slice(hg * HG_D, (hg + 1) * HG_D)
        in_maps.append({
            "xt": np.ascontiguousarray(np.asarray(x[b]).T).astype(BF16),
            "wq": np.ascontiguousarray(np.asarray(Wq)[:, sl]).astype(BF16),
            "wk": np.ascontiguousarray(np.asarray(Wk)[:, sl]).astype(BF16),
            "wv": np.ascontiguousarray(np.asarray(Wv)[:, sl]).astype(BF16),
            "wo": np.ascontiguousarray(np.asarray(Wo)[sl, :]).astype(BF16),
            "bqt": np.ascontiguousarray(
                np.asarray(bq, np.float32)[sl].reshape(OT, 128).T),
            "bkt": np.ascontiguousarray(
                np.asarray(bk, np.float32)[sl].reshape(OT, 128).T),
            "bvr": np.asarray(bv, np.float32)[sl].reshape(1, HG_D).astype(BF16),
            "bor": (np.asarray(bo, np.float32) if hg == 0
                    else np.zeros(D, np.float32)).reshape(1, D).astype(BF16),
        })
    return in_maps


def run_cores(in_maps, trace=False, with_bv=True, with_bo=True):
    try:
        import ntff_shim
        ntff_shim.install()
    except Exception:
        pass
    from concourse.bass_utils import run_bass_kernel_spmd

    nc = get_nc(with_bv, with_bo)
    return run_bass_kernel_spmd(nc, in_maps, list(range(N_CORES)), trace=trace)


def combine(results):
    y = np.empty((4, S, D), np.float32)
    for b in range(4):
        y[b] = results[2 * b]["y"] + results[2 * b + 1]["y"]
    return y


def kernel(x, Wq, bq, Wk, bk, Wv, bv, Wo, bo):
    in_maps = make_in_maps(x, Wq, bq, Wk, bk, Wv, bv, Wo, bo)
    with_bv = bool(np.any(np.asarray(bv)))
    with_bo = bool(np.any(np.asarray(bo)))
    res = run_cores(in_maps, trace=False, with_bv=with_bv, with_bo=with_bo)
    return combine(res.results)


# revision 28
# speedup vs baseline: 1.0822x; 1.0103x over previous
"""Multi-head attention (B=4, S=2048, D=1024, H=16, Hd=64) on 8 trn2 cores.

Sharding: core c = (batch b = c // 2, head-group hg = c % 2). Each core
computes attention for 8 heads of one batch and the corresponding slice of
the output projection; host sums the two partial outputs per batch.

Per-core layout (all matmuls bf16 with fp32 PSUM accumulation):
  xt   = x[b].T                    [D=1024, S=2048]  (lhsT/rhs K-major)
  qT/kT = (Wslice.T @ .. )         [512, 2048]  d-major, 4 pair-tiles of 128
  v    = x @ Wv_slice              [2048, 512]  s-major
  per head: scoresT[k,q] tile = kT.T-block @ qT  -> exp (ScalarE, scale=1/8)
            outT[d,q] += v-block.T @ expT ; denom[q] += ones.T @ expT
  normalize: outT *= broadcast(1/denom) via K=1 ones-matmul
  y = outT.T-blocks @ Wo_slice + bo   [2048, 1024] fp32 partial
"""

import numpy as np
import ml_dtypes

S = 2048
D = 1024
HG_D = 512          # head dims per core (8 heads x 64)
NH = 8              # heads per core
KT = S // 128       # 16 k-tiles
DT = D // 128       # 8 contraction tiles for QKV
ST = S // 128       # 16 s-tiles
OT = HG_D // 128    # 4 contraction tiles for O-proj / pair tiles
N_CORES = 8

BF16 = ml_dtypes.bfloat16

_CACHED_NC = {}


def _build_nc(with_bv=True, with_bo=True):
    import concourse.bass as bass  # noqa: F401
    import concourse.mybir as mybir
    import concourse.tile as tile
    from concourse import bacc

    f32 = mybir.dt.float32
    bf16 = mybir.dt.bfloat16
    fp16 = mybir.dt.float16
    Exp = mybir.ActivationFunctionType.Exp

    nc = bacc.Bacc("TRN2", target_bir_lowering=False, debug=False,
                   num_devices=N_CORES)

    xt_d = nc.dram_tensor("xt", [D, S], bf16, kind="ExternalInput")
    wq_d = nc.dram_tensor("wq", [D, HG_D], bf16, kind="ExternalInput")
    wk_d = nc.dram_tensor("wk", [D, HG_D], bf16, kind="ExternalInput")
    wv_d = nc.dram_tensor("wv", [D, HG_D], bf16, kind="ExternalInput")
    wo_d = nc.dram_tensor("wo", [HG_D, D], bf16, kind="ExternalInput")
    bq_d = nc.dram_tensor("bqt", [128, OT], f32, kind="ExternalInput")
    bk_d = nc.dram_tensor("bkt", [128, OT], f32, kind="ExternalInput")
    bv_d = nc.dram_tensor("bvr", [1, HG_D], bf16, kind="ExternalInput")
    bo_d = nc.dram_tensor("bor", [1, D], bf16, kind="ExternalInput")
    y_d = nc.dram_tensor("y", [S, D], f32, kind="ExternalOutput")

    with tile.TileContext(nc) as tc:
        with (
            tc.tile_pool(name="cpool", bufs=1) as cpool,
            tc.tile_pool(name="wpool", bufs=2) as wpool,
            tc.tile_pool(name="pspool", bufs=3, space="PSUM") as pspool,
            tc.tile_pool(name="popool", bufs=2, space="PSUM") as popool,
        ):
            # ---- persistent SBUF tiles ----
            xt_sb = cpool.tile([128, DT, S], bf16, name="xt_sb")
            wq_sb = cpool.tile([128, DT, HG_D], bf16, name="wq_sb")
            wk_sb = cpool.tile([128, DT, HG_D], bf16, name="wk_sb")
            wv_sb = cpool.tile([128, DT, HG_D], bf16, name="wv_sb")
            wo_sb = cpool.tile([128, OT, D], bf16, name="wo_sb")
            bq_sb = cpool.tile([128, OT], f32, name="bq_sb")
            bk_sb = cpool.tile([128, OT], f32, name="bk_sb")
            bvr_sb = cpool.tile([1, HG_D], bf16, name="bvr_sb")
            bor_sb = cpool.tile([1, D], bf16, name="bor_sb")
            ones_t = cpool.tile([128, 128], bf16, name="ones_t")
            qT_sb = cpool.tile([128, OT, S], bf16, name="qT_sb")
            kT_sb = cpool.tile([128, OT, S], bf16, name="kT_sb")
            # v with a trailing ones column per head: attnv lhsT [128, 65]
            # whose 65th output row accumulates the softmax denominator.
            v_sb = cpool.tile([128, ST, NH, 65], bf16, name="v_sb")
            aoT_sb = cpool.tile([128, OT, S], bf16, name="aoT_sb")

            # ---- loads ----
            for k in range(DT):
                nc.sync.dma_start(out=xt_sb[:, k, :], in_=xt_d[k * 128:(k + 1) * 128, :])
                nc.sync.dma_start(out=wq_sb[:, k, :], in_=wq_d[k * 128:(k + 1) * 128, :])
                nc.sync.dma_start(out=wk_sb[:, k, :], in_=wk_d[k * 128:(k + 1) * 128, :])
                nc.sync.dma_start(out=wv_sb[:, k, :], in_=wv_d[k * 128:(k + 1) * 128, :])
            for k in range(OT):
                nc.sync.dma_start(out=wo_sb[:, k, :], in_=wo_d[k * 128:(k + 1) * 128, :])
            nc.sync.dma_start(out=bq_sb[:], in_=bq_d[:])
            nc.sync.dma_start(out=bk_sb[:], in_=bk_d[:])
            nc.sync.dma_start(out=bvr_sb[:], in_=bv_d[:])
            nc.sync.dma_start(out=bor_sb[:], in_=bo_d[:])
            nc.gpsimd.memset(ones_t[:], 1.0)

            # ---- projections: qT/kT [d-major], bias via per-partition scalar ----
            def emit_qk_group(w_sb, b_sb, out_sb, p, jc):
                pq = pspool.tile([128, 512], f32, tag="ps", name="pq")
                for k in range(DT):
                    nc.tensor.matmul(
                        pq[:],
                        w_sb[:, k, p * 128:(p + 1) * 128],
                        xt_sb[:, k, jc * 512:(jc + 1) * 512],
                        start=(k == 0), stop=(k == DT - 1),
                    )
                nc.vector.tensor_scalar_add(
                    out_sb[:, p, jc * 512:(jc + 1) * 512], pq[:], b_sb[:, p:p + 1])

            for w_sb, b_sb, out_sb in ((wq_sb, bq_sb, qT_sb), (wk_sb, bk_sb, kT_sb)):
                for p in range(OT):
                    for jc in range(4):
                        emit_qk_group(w_sb, b_sb, out_sb, p, jc)

            # ---- v projection [s-major], bias via K=1 ones matmul ----
            nc.vector.memset(v_sb[:], 1.0)
            for st in range(ST):
                pv = pspool.tile([128, 512], f32, tag="ps", name="pv")
                for k in range(DT):
                    nc.tensor.matmul(
                        pv[:],
                        xt_sb[:, k, st * 128:(st + 1) * 128],
                        wv_sb[:, k, :],
                        start=(k == 0), stop=(not with_bv and k == DT - 1),
                    )
                if with_bv:
                    nc.tensor.matmul(pv[:], ones_t[0:1, 0:128], bvr_sb[0:1, :],
                                     start=False, stop=True)
                nc.vector.tensor_copy(
                    v_sb[:, st, :, 0:64],
                    pv.rearrange("p (h c) -> p h c", c=64))

            # ---- attention, one head PAIR per chunk ----
            # Both heads of a pair are processed together so consecutive
            # scores matmuls alternate PE row halves (rows 0-63 / 64-127),
            # which lets the hardware overlap them (~2x scores throughput).
            # Normalization (broadcast of 1/denom + scale) is deferred by one
            # chunk so the PE never waits on the slow DVE reciprocal.
            pending = []

            def emit_oproj_group(st):
                yt = wpool.tile([128, D], f32, tag="y", name="yt")
                for l in range(2):
                    py = pspool.tile([128, 512], f32, tag="ps", name="py")
                    for kt in range(OT):
                        nc.tensor.matmul(
                            py[:],
                            aoT_sb[:, kt, st * 128:(st + 1) * 128],
                            wo_sb[:, kt, l * 512:(l + 1) * 512],
                            start=(kt == 0), stop=(not with_bo and kt == OT - 1),
                        )
                    if with_bo:
                        nc.tensor.matmul(py[:], ones_t[0:1, 0:128],
                                         bor_sb[0:1, l * 512:(l + 1) * 512],
                                         start=False, stop=True)
                    nc.vector.tensor_copy(yt[:, l * 512:(l + 1) * 512], py[:])
                nc.sync.dma_start(out=y_d[st * 128:(st + 1) * 128, :], in_=yt[:])

            def flush_normalize():
                off2, p2, jb2, rb2, row = pending.pop(0)
                bt = pspool.tile([128, 512], f32, tag="ps", name="bt")
                nc.tensor.matmul(
                    bt[off2:off2 + 64, :],
                    ones_t[row:row + 1, 0:64],
                    rb2[row:row + 1, :],
                    start=True, stop=True,
                )
                nc.vector.tensor_mul(
                    aoT_sb[off2:off2 + 64, p2, jb2],
                    aoT_sb[off2:off2 + 64, p2, jb2],
                    bt[off2:off2 + 64, :])

            oproj_done = set()
            for p in range(OT):
                for c in range(4):
                    jb = slice(c * 512, (c + 1) * 512)
                    while len(pending) > 2:
                        flush_normalize()
                    otA = popool.tile([65, 512], f32, tag="po", name="otA")
                    otB = popool.tile([65, 512], f32, tag="po", name="otB")
                    for i in range(KT):
                        # combined scores psum: head A in cols 0-511 (rows
                        # 0-63 of the PE array), head B in cols 512-1023
                        # (rows 64-127) -- consecutive matmuls alternate PE
                        # row halves so the array overlaps them.
                        stt = pspool.tile([128, 1024], f32, tag="ps",
                                          name="stt")
                        for off in (0, 64):
                            nc.tensor.matmul(
                                stt[:, off * 8:off * 8 + 512],
                                kT_sb[off:off + 64, p, i * 128:(i + 1) * 128],
                                qT_sb[off:off + 64, p, jb],
                                start=True, stop=True,
                            )
                        et = wpool.tile([128, 1024], bf16, tag="exp", bufs=4,
                                        name="et")
                        nc.scalar.activation(et[:], stt[:], Exp, scale=0.125)
                        for ot, hh in ((otA, 0), (otB, 1)):
                            nc.tensor.matmul(
                                ot[:],
                                v_sb[:, i, 2 * p + hh, :],
                                et[:, hh * 512:(hh + 1) * 512],
                                start=(i == 0), stop=(i == KT - 1),
                            )
                    # Drain attn rows (cross-partition for the odd head) first
                    # so the PSUM tiles free fast, then gather the denominator
                    # rows to partitions 64 (A) / 32 (B) and batch-reciprocal.
                    nc.vector.tensor_copy(aoT_sb[0:64, p, jb], otA[0:64, :])
                    nc.vector.tensor_copy(aoT_sb[64:128, p, jb], otB[0:64, :])
                    den = wpool.tile([97, 512], f32, tag="den", bufs=2,
                                     name="den")
                    nc.vector.memset(den[64:97, :], 1.0)
                    nc.vector.tensor_copy(den[64:65, :], otA[64:65, :])
                    nc.vector.tensor_copy(den[96:97, :], otB[64:65, :])
                    rf = wpool.tile([97, 512], f32, tag="rf", name="rf")
                    rb = wpool.tile([65, 512], bf16, tag="rb", bufs=3,
                                    name="rb")
                    nc.vector.reciprocal(rf[64:97, :], den[64:97, :])
                    nc.vector.tensor_copy(rb[64:65, :], rf[64:65, :])
                    nc.vector.tensor_copy(rb[32:33, :], rf[96:97, :])
                    pending.append((0, p, jb, rb, 64))
                    pending.append((64, p, jb, rb, 32))
            while pending:
                flush_normalize()

            # ---- remaining output projection ----
            for st in range(ST):
                if st not in oproj_done:
                    emit_oproj_group(st)

    nc.compile()
    return nc


def get_nc(with_bv=True, with_bo=True):
    key = (with_bv, with_bo)
    if key not in _CACHED_NC:
        _CACHED_NC[key] = _build_nc(*key)
    return _CACHED_NC[key]


def make_in_maps(x, Wq, bq, Wk, bk, Wv, bv, Wo, bo):
    x = np.asarray(x, dtype=np.float32)
    in_maps = []
    for c in range(N_CORES):
        b, hg = c // 2, c % 2
        sl = slice(hg * HG_D, (hg + 1) * HG_D)
        in_maps.append({
            "xt": np.ascontiguousarray(np.asarray(x[b]).T).astype(BF16),
            "wq": np.ascontiguousarray(np.asarray(Wq)[:, sl]).astype(BF16),
            "wk": np.ascontiguousarray(np.asarray(Wk)[:, sl]).astype(BF16),
            "wv": np.ascontiguousarray(np.asarray(Wv)[:, sl]).astype(BF16),
            "wo": np.ascontiguousarray(np.asarray(Wo)[sl, :]).astype(BF16),
            "bqt": np.ascontiguousarray(
                np.asarray(bq, np.float32)[sl].reshape(OT, 128).T),
            "bkt": np.ascontiguousarray(
                np.asarray(bk, np.float32)[sl].reshape(OT, 128).T),
            "bvr": np.asarray(bv, np.float32)[sl].reshape(1, HG_D).astype(BF16),
            "bor": (np.asarray(bo, np.float32) if hg == 0
                    else np.zeros(D, np.float32)).reshape(1, D).astype(BF16),
        })
    return in_maps


def run_cores(in_maps, trace=False, with_bv=True, with_bo=True):
    try:
        import ntff_shim
        ntff_shim.install()
    except Exception:
        pass
    from concourse.bass_utils import run_bass_kernel_spmd

    nc = get_nc(with_bv, with_bo)
    return run_bass_kernel_spmd(nc, in_maps, list(range(N_CORES)), trace=trace)


def combine(results):
    y = np.empty((4, S, D), np.float32)
    for b in range(4):
        y[b] = results[2 * b]["y"] + results[2 * b + 1]["y"]
    return y


def kernel(x, Wq, bq, Wk, bk, Wv, bv, Wo, bo):
    in_maps = make_in_maps(x, Wq, bq, Wk, bk, Wv, bv, Wo, bo)
    with_bv = bool(np.any(np.asarray(bv)))
    with_bo = bool(np.any(np.asarray(bo)))
    res = run_cores(in_maps, trace=False, with_bv=with_bv, with_bo=with_bo)
    return combine(res.results)


# revision 30
# speedup vs baseline: 1.0874x; 1.0048x over previous
"""Multi-head attention (B=4, S=2048, D=1024, H=16, Hd=64) on 8 trn2 cores.

Sharding: core c = (batch b = c // 2, head-group hg = c % 2). Each core
computes attention for 8 heads of one batch and the corresponding slice of
the output projection; host sums the two partial outputs per batch.

Per-core layout (all matmuls bf16 with fp32 PSUM accumulation):
  xt   = x[b].T                    [D=1024, S=2048]  (lhsT/rhs K-major)
  qT/kT = (Wslice.T @ .. )         [512, 2048]  d-major, 4 pair-tiles of 128
  v    = x @ Wv_slice              [2048, 512]  s-major
  per head: scoresT[k,q] tile = kT.T-block @ qT  -> exp (ScalarE, scale=1/8)
            outT[d,q] += v-block.T @ expT ; denom[q] += ones.T @ expT
  normalize: outT *= broadcast(1/denom) via K=1 ones-matmul
  y = outT.T-blocks @ Wo_slice + bo   [2048, 1024] fp32 partial
"""

import numpy as np
import ml_dtypes

S = 2048
D = 1024
HG_D = 512          # head dims per core (8 heads x 64)
NH = 8              # heads per core
KT = S // 128       # 16 k-tiles
DT = D // 128       # 8 contraction tiles for QKV
ST = S // 128       # 16 s-tiles
OT = HG_D // 128    # 4 contraction tiles for O-proj / pair tiles
N_CORES = 8

BF16 = ml_dtypes.bfloat16

_CACHED_NC = {}


def _build_nc(with_bv=True, with_bo=True):
    import concourse.bass as bass  # noqa: F401
    import concourse.mybir as mybir
    import concourse.tile as tile
    from concourse import bacc

    f32 = mybir.dt.float32
    bf16 = mybir.dt.bfloat16
    fp16 = mybir.dt.float16
    Exp = mybir.ActivationFunctionType.Exp

    nc = bacc.Bacc("TRN2", target_bir_lowering=False, debug=False,
                   num_devices=N_CORES)

    xt_d = nc.dram_tensor("xt", [D, S], bf16, kind="ExternalInput")
    wq_d = nc.dram_tensor("wq", [D, HG_D], bf16, kind="ExternalInput")
    wk_d = nc.dram_tensor("wk", [D, HG_D], bf16, kind="ExternalInput")
    wv_d = nc.dram_tensor("wv", [D, HG_D], bf16, kind="ExternalInput")
    wo_d = nc.dram_tensor("wo", [HG_D, D], bf16, kind="ExternalInput")
    bq_d = nc.dram_tensor("bqt", [128, OT], f32, kind="ExternalInput")
    bk_d = nc.dram_tensor("bkt", [128, OT], f32, kind="ExternalInput")
    bv_d = nc.dram_tensor("bvr", [1, HG_D], bf16, kind="ExternalInput")
    bo_d = nc.dram_tensor("bor", [1, D], bf16, kind="ExternalInput")
    y_d = nc.dram_tensor("y", [S, D], f32, kind="ExternalOutput")

    with tile.TileContext(nc) as tc:
        with (
            tc.tile_pool(name="cpool", bufs=1) as cpool,
            tc.tile_pool(name="wpool", bufs=2) as wpool,
            tc.tile_pool(name="pspool", bufs=3, space="PSUM") as pspool,
            tc.tile_pool(name="popool", bufs=2, space="PSUM") as popool,
        ):
            # ---- persistent SBUF tiles ----
            xt_sb = cpool.tile([128, DT, S], bf16, name="xt_sb")
            wq_sb = cpool.tile([128, DT, HG_D], bf16, name="wq_sb")
            wk_sb = cpool.tile([128, DT, HG_D], bf16, name="wk_sb")
            wv_sb = cpool.tile([128, DT, HG_D], bf16, name="wv_sb")
            wo_sb = cpool.tile([128, OT, D], bf16, name="wo_sb")
            bq_sb = cpool.tile([128, OT], f32, name="bq_sb")
            bk_sb = cpool.tile([128, OT], f32, name="bk_sb")
            bvr_sb = cpool.tile([1, HG_D], bf16, name="bvr_sb")
            bor_sb = cpool.tile([1, D], bf16, name="bor_sb")
            ones_t = cpool.tile([128, 128], bf16, name="ones_t")
            qT_sb = cpool.tile([128, OT, S], bf16, name="qT_sb")
            kT_sb = cpool.tile([128, OT, S], bf16, name="kT_sb")
            # v with a trailing ones column per head: attnv lhsT [128, 65]
            # whose 65th output row accumulates the softmax denominator.
            v_sb = cpool.tile([128, ST, NH, 65], bf16, name="v_sb")
            aoT_sb = cpool.tile([128, OT, S], bf16, name="aoT_sb")

            # ---- loads ----
            for k in range(DT):
                nc.sync.dma_start(out=xt_sb[:, k, :], in_=xt_d[k * 128:(k + 1) * 128, :])
                nc.sync.dma_start(out=wq_sb[:, k, :], in_=wq_d[k * 128:(k + 1) * 128, :])
                nc.sync.dma_start(out=wk_sb[:, k, :], in_=wk_d[k * 128:(k + 1) * 128, :])
                nc.sync.dma_start(out=wv_sb[:, k, :], in_=wv_d[k * 128:(k + 1) * 128, :])
            for k in range(OT):
                nc.sync.dma_start(out=wo_sb[:, k, :], in_=wo_d[k * 128:(k + 1) * 128, :])
            nc.sync.dma_start(out=bq_sb[:], in_=bq_d[:])
            nc.sync.dma_start(out=bk_sb[:], in_=bk_d[:])
            nc.sync.dma_start(out=bvr_sb[:], in_=bv_d[:])
            nc.sync.dma_start(out=bor_sb[:], in_=bo_d[:])
            nc.gpsimd.memset(ones_t[:], 1.0)

            # ---- projections: qT/kT [d-major], bias via per-partition scalar ----
            def emit_qk_group(w_sb, b_sb, out_sb, p, jc):
                pq = pspool.tile([128, 512], f32, tag="ps", name="pq")
                for k in range(DT):
                    nc.tensor.matmul(
                        pq[:],
                        w_sb[:, k, p * 128:(p + 1) * 128],
                        xt_sb[:, k, jc * 512:(jc + 1) * 512],
                        start=(k == 0), stop=(k == DT - 1),
                    )
                nc.vector.tensor_scalar_add(
                    out_sb[:, p, jc * 512:(jc + 1) * 512], pq[:], b_sb[:, p:p + 1])

            for w_sb, b_sb, out_sb in ((wq_sb, bq_sb, qT_sb), (wk_sb, bk_sb, kT_sb)):
                for p in range(OT):
                    for jc in range(4):
                        emit_qk_group(w_sb, b_sb, out_sb, p, jc)

            # ---- v projection [s-major], bias via K=1 ones matmul ----
            nc.vector.memset(v_sb[:], 1.0)
            for st in range(ST):
                pv = pspool.tile([128, 512], f32, tag="ps", name="pv")
                for k in range(DT):
                    nc.tensor.matmul(
                        pv[:],
                        xt_sb[:, k, st * 128:(st + 1) * 128],
                        wv_sb[:, k, :],
                        start=(k == 0), stop=(not with_bv and k == DT - 1),
                    )
                if with_bv:
                    nc.tensor.matmul(pv[:], ones_t[0:1, 0:128], bvr_sb[0:1, :],
                                     start=False, stop=True)
                nc.vector.tensor_copy(
                    v_sb[:, st, :, 0:64],
                    pv.rearrange("p (h c) -> p h c", c=64))

            # ---- attention, one head PAIR per chunk ----
            # Both heads of a pair are processed together so consecutive
            # scores matmuls alternate PE row halves (rows 0-63 / 64-127),
            # which lets the hardware overlap them (~2x scores throughput).
            # Normalization (broadcast of 1/denom + scale) is deferred by one
            # chunk so the PE never waits on the slow DVE reciprocal.
            pending = []

            def emit_oproj_group(st):
                yt = wpool.tile([128, D], f32, tag="y", name="yt")
                for l in range(2):
                    py = pspool.tile([128, 512], f32, tag="ps", name="py")
                    for kt in range(OT):
                        nc.tensor.matmul(
                            py[:],
                            aoT_sb[:, kt, st * 128:(st + 1) * 128],
                            wo_sb[:, kt, l * 512:(l + 1) * 512],
                            start=(kt == 0), stop=(not with_bo and kt == OT - 1),
                        )
                    if with_bo:
                        nc.tensor.matmul(py[:], ones_t[0:1, 0:128],
                                         bor_sb[0:1, l * 512:(l + 1) * 512],
                                         start=False, stop=True)
                    nc.vector.tensor_copy(yt[:, l * 512:(l + 1) * 512], py[:])
                nc.sync.dma_start(out=y_d[st * 128:(st + 1) * 128, :], in_=yt[:])

            def flush_normalize():
                off2, p2, jb2, rb2, row = pending.pop(0)
                bt = pspool.tile([128, 512], f32, tag="ps", name="bt")
                nc.tensor.matmul(
                    bt[off2:off2 + 64, :],
                    ones_t[row:row + 1, 0:64],
                    rb2[row:row + 1, :],
                    start=True, stop=True,
                )
                nc.vector.tensor_mul(
                    aoT_sb[off2:off2 + 64, p2, jb2],
                    aoT_sb[off2:off2 + 64, p2, jb2],
                    bt[off2:off2 + 64, :])

            oproj_done = set()
            for p in range(OT):
                for c in range(4):
                    jb = slice(c * 512, (c + 1) * 512)
                    while len(pending) > 2:
                        flush_normalize()
                    otA = popool.tile([65, 512], f32, tag="po", name="otA")
                    otB = popool.tile([65, 512], f32, tag="po", name="otB")
                    for i in range(KT):
                        # combined scores psum: head A in cols 0-511 (rows
                        # 0-63 of the PE array), head B in cols 512-1023
                        # (rows 64-127) -- consecutive matmuls alternate PE
                        # row halves so the array overlaps them.
                        stt = pspool.tile([128, 1024], f32, tag="ps",
                                          name="stt")
                        for off in (0, 64):
                            nc.tensor.matmul(
                                stt[:, off * 8:off * 8 + 512],
                                kT_sb[off:off + 64, p, i * 128:(i + 1) * 128],
                                qT_sb[off:off + 64, p, jb],
                                start=True, stop=True,
                            )
                        et = wpool.tile([128, 1024], bf16, tag="exp", bufs=4,
                                        name="et")
                        nc.scalar.activation(et[:], stt[:], Exp, scale=0.125)
                        for ot, hh in ((otA, 0), (otB, 1)):
                            nc.tensor.matmul(
                                ot[:],
                                v_sb[:, i, 2 * p + hh, :],
                                et[:, hh * 512:(hh + 1) * 512],
                                start=(i == 0), stop=(i == KT - 1),
                            )
                    # Drain attn rows (cross-partition for the odd head) first
                    # so the PSUM tiles free fast, then gather the denominator
                    # rows to partitions 64 (A) / 32 (B) and batch-reciprocal.
                    nc.vector.tensor_copy(aoT_sb[0:64, p, jb], otA[0:64, :])
                    nc.vector.tensor_copy(aoT_sb[64:128, p, jb], otB[0:64, :])
                    den = wpool.tile([97, 512], f32, tag="den", bufs=2,
                                     name="den")
                    nc.vector.memset(den[64:97, :], 1.0)
                    nc.vector.tensor_copy(den[64:65, :], otA[64:65, :])
                    nc.vector.tensor_copy(den[96:97, :], otB[64:65, :])
                    rf = wpool.tile([97, 512], f32, tag="rf", name="rf")
                    rb = wpool.tile([65, 512], bf16, tag="rb", bufs=3,
                                    name="rb")
                    nc.vector.reciprocal(rf[64:97, :], den[64:97, :])
                    nc.vector.tensor_copy(rb[64:65, :], rf[64:65, :])
                    nc.vector.tensor_copy(rb[32:33, :], rf[96:97, :])
                    pending.append((0, p, jb, rb, 64))
                    pending.append((64, p, jb, rb, 32))
            while pending:
                flush_normalize()

            # ---- remaining output projection ----
            for st in range(ST):
                if st not in oproj_done:
                    emit_oproj_group(st)

    nc.compile()
    return nc


def get_nc(with_bv=True, with_bo=True):
    key = (with_bv, with_bo)
    if key not in _CACHED_NC:
        _CACHED_NC[key] = _build_nc(*key)
    return _CACHED_NC[key]


def make_in_maps(x, Wq, bq, Wk, bk, Wv, bv, Wo, bo):
    x = np.asarray(x, dtype=np.float32)
    in_maps = []
    for c in range(N_CORES):
        b, hg = c // 2, c % 2
        sl = slice(hg * HG_D, (hg + 1) * HG_D)
        in_maps.append({
            "xt": np.ascontiguousarray(np.asarray(x[b]).T).astype(BF16),
            "wq": np.ascontiguousarray(np.asarray(Wq)[:, sl]).astype(BF16),
            "wk": np.ascontiguousarray(np.asarray(Wk)[:, sl]).astype(BF16),
            "wv": np.ascontiguousarray(np.asarray(Wv)[:, sl]).astype(BF16),
            "wo": np.ascontiguousarray(np.asarray(Wo)[sl, :]).astype(BF16),
            "bqt": np.ascontiguousarray(
                np.asarray(bq, np.float32)[sl].reshape(OT, 128).T),
            "bkt": np.ascontiguousarray(
                np.asarray(bk, np.float32)[sl].reshape(OT, 128).T),
            "bvr": np.asarray(bv, np.float32)[sl].reshape(1, HG_D).astype(BF16),
            "bor": (np.asarray(bo, np.float32) if hg == 0
                    else np.zeros(D, np.float32)).reshape(1, D).astype(BF16),
        })
    return in_maps


def run_cores(in_maps, trace=False, with_bv=True, with_bo=True):
    try:
        import ntff_shim
        ntff_shim.install()
    except Exception:
        pass
    from concourse.bass_utils import run_bass_kernel_spmd

    nc = get_nc(with_bv, with_bo)
    return run_bass_kernel_spmd(nc, in_maps, list(range(N_CORES)), trace=trace)


def combine(results):
    y = np.empty((4, S, D), np.float32)
    for b in range(4):
        y[b] = results[2 * b]["y"] + results[2 * b + 1]["y"]
    return y


def kernel(x, Wq, bq, Wk, bk, Wv, bv, Wo, bo):
    in_maps = make_in_maps(x, Wq, bq, Wk, bk, Wv, bv, Wo, bo)
    with_bv = bool(np.any(np.asarray(bv)))
    with_bo = bool(np.any(np.asarray(bo)))
    res = run_cores(in_maps, trace=False, with_bv=with_bv, with_bo=with_bo)
    return combine(res.results)


# revision 32
# speedup vs baseline: 1.0969x; 1.0087x over previous
"""Multi-head attention (B=4, S=2048, D=1024, H=16, Hd=64) on 8 trn2 cores.

Sharding: core c = (batch b = c // 2, head-group hg = c % 2). Each core
computes attention for 8 heads of one batch and the corresponding slice of
the output projection; host sums the two partial outputs per batch.

Per-core layout (all matmuls bf16 with fp32 PSUM accumulation):
  xt   = x[b].T                    [D=1024, S=2048]  (lhsT/rhs K-major)
  qT/kT = (Wslice.T @ .. )         [512, 2048]  d-major, 4 pair-tiles of 128
  v    = x @ Wv_slice              [2048, 512]  s-major
  per head: scoresT[k,q] tile = kT.T-block @ qT  -> exp (ScalarE, scale=1/8)
            outT[d,q] += v-block.T @ expT ; denom[q] += ones.T @ expT
  normalize: outT *= broadcast(1/denom) via K=1 ones-matmul
  y = outT.T-blocks @ Wo_slice + bo   [2048, 1024] fp32 partial
"""

import numpy as np
import ml_dtypes

S = 2048
D = 1024
HG_D = 512          # head dims per core (8 heads x 64)
NH = 8              # heads per core
KT = S // 128       # 16 k-tiles
DT = D // 128       # 8 contraction tiles for QKV
ST = S // 128       # 16 s-tiles
OT = HG_D // 128    # 4 contraction tiles for O-proj / pair tiles
N_CORES = 8

BF16 = ml_dtypes.bfloat16

_CACHED_NC = {}


def _build_nc(with_bv=True, with_bo=True):
    import concourse.bass as bass  # noqa: F401
    import concourse.mybir as mybir
    import concourse.tile as tile
    from concourse import bacc

    f32 = mybir.dt.float32
    bf16 = mybir.dt.bfloat16
    fp16 = mybir.dt.float16
    Exp = mybir.ActivationFunctionType.Exp

    nc = bacc.Bacc("TRN2", target_bir_lowering=False, debug=False,
                   num_devices=N_CORES)

    xt_d = nc.dram_tensor("xt", [D, S], bf16, kind="ExternalInput")
    wq_d = nc.dram_tensor("wq", [D, HG_D], bf16, kind="ExternalInput")
    wk_d = nc.dram_tensor("wk", [D, HG_D], bf16, kind="ExternalInput")
    wv_d = nc.dram_tensor("wv", [D, HG_D], bf16, kind="ExternalInput")
    wo_d = nc.dram_tensor("wo", [HG_D, D], bf16, kind="ExternalInput")
    bq_d = nc.dram_tensor("bqt", [128, OT], f32, kind="ExternalInput")
    bk_d = nc.dram_tensor("bkt", [128, OT], f32, kind="ExternalInput")
    bv_d = nc.dram_tensor("bvr", [1, HG_D], bf16, kind="ExternalInput")
    bo_d = nc.dram_tensor("bor", [1, D], bf16, kind="ExternalInput")
    y_d = nc.dram_tensor("y", [S, D], f32, kind="ExternalOutput")

    with tile.TileContext(nc) as tc:
        with (
            tc.tile_pool(name="cpool", bufs=1) as cpool,
            tc.tile_pool(name="wpool", bufs=2) as wpool,
            tc.tile_pool(name="pspool", bufs=3, space="PSUM") as pspool,
            tc.tile_pool(name="popool", bufs=2, space="PSUM") as popool,
        ):
            # ---- persistent SBUF tiles ----
            xt_sb = cpool.tile([128, DT, S], bf16, name="xt_sb")
            wq_sb = cpool.tile([128, DT, HG_D], bf16, name="wq_sb")
            wk_sb = cpool.tile([128, DT, HG_D], bf16, name="wk_sb")
            wv_sb = cpool.tile([128, DT, HG_D], bf16, name="wv_sb")
            wo_sb = cpool.tile([128, OT, D], bf16, name="wo_sb")
            bq_sb = cpool.tile([128, OT], f32, name="bq_sb")
            bk_sb = cpool.tile([128, OT], f32, name="bk_sb")
            bvr_sb = cpool.tile([1, HG_D], bf16, name="bvr_sb")
            bor_sb = cpool.tile([1, D], bf16, name="bor_sb")
            ones_t = cpool.tile([128, 128], bf16, name="ones_t")
            qT_sb = cpool.tile([128, OT, S], bf16, name="qT_sb")
            kT_sb = cpool.tile([128, OT, S], bf16, name="kT_sb")
            # v with a trailing ones column per head: attnv lhsT [128, 65]
            # whose 65th output row accumulates the softmax denominator.
            v_sb = cpool.tile([128, ST, NH, 65], bf16, name="v_sb")
            aoT_sb = cpool.tile([128, OT, S], bf16, name="aoT_sb")

            # ---- loads ----
            for k in range(DT):
                nc.sync.dma_start(out=xt_sb[:, k, :], in_=xt_d[k * 128:(k + 1) * 128, :])
                nc.sync.dma_start(out=wq_sb[:, k, :], in_=wq_d[k * 128:(k + 1) * 128, :])
                nc.sync.dma_start(out=wk_sb[:, k, :], in_=wk_d[k * 128:(k + 1) * 128, :])
                nc.sync.dma_start(out=wv_sb[:, k, :], in_=wv_d[k * 128:(k + 1) * 128, :])
            for k in range(OT):
                nc.sync.dma_start(out=wo_sb[:, k, :], in_=wo_d[k * 128:(k + 1) * 128, :])
            nc.sync.dma_start(out=bq_sb[:], in_=bq_d[:])
            nc.sync.dma_start(out=bk_sb[:], in_=bk_d[:])
            nc.sync.dma_start(out=bvr_sb[:], in_=bv_d[:])
            nc.sync.dma_start(out=bor_sb[:], in_=bo_d[:])
            nc.gpsimd.memset(ones_t[:], 1.0)

            # ---- projections: qT/kT [d-major], bias via per-partition scalar ----
            def emit_qk_group(w_sb, b_sb, out_sb, p, jc):
                pq = pspool.tile([128, 512], f32, tag="ps", name="pq")
                for k in range(DT):
                    nc.tensor.matmul(
                        pq[:],
                        w_sb[:, k, p * 128:(p + 1) * 128],
                        xt_sb[:, k, jc * 512:(jc + 1) * 512],
                        start=(k == 0), stop=(k == DT - 1),
                    )
                nc.vector.tensor_scalar_add(
                    out_sb[:, p, jc * 512:(jc + 1) * 512], pq[:], b_sb[:, p:p + 1])

            for w_sb, b_sb, out_sb in ((wq_sb, bq_sb, qT_sb), (wk_sb, bk_sb, kT_sb)):
                for p in range(OT):
                    for jc in range(4):
                        emit_qk_group(w_sb, b_sb, out_sb, p, jc)

            # ---- v projection [s-major], bias via K=1 ones matmul ----
            nc.vector.memset(v_sb[:], 1.0)
            for st in range(ST):
                pv = pspool.tile([128, 512], f32, tag="ps", name="pv")
                for k in range(DT):
                    nc.tensor.matmul(
                        pv[:],
                        xt_sb[:, k, st * 128:(st + 1) * 128],
                        wv_sb[:, k, :],
                        start=(k == 0), stop=(not with_bv and k == DT - 1),
                    )
                if with_bv:
                    nc.tensor.matmul(pv[:], ones_t[0:1, 0:128], bvr_sb[0:1, :],
                                     start=False, stop=True)
                nc.vector.tensor_copy(
                    v_sb[:, st, :, 0:64],
                    pv.rearrange("p (h c) -> p h c", c=64))

            # ---- attention, one head PAIR per chunk ----
            # Both heads of a pair are processed together so consecutive
            # scores matmuls alternate PE row halves (rows 0-63 / 64-127),
            # which lets the hardware overlap them (~2x scores throughput).
            # Normalization (broadcast of 1/denom + scale) is deferred by one
            # chunk so the PE never waits on the slow DVE reciprocal.
            pending = []

            def emit_oproj_group(st):
                yt = wpool.tile([128, D], f32, tag="y", name="yt")
                for l in range(2):
                    py = pspool.tile([128, 512], f32, tag="ps", name="py")
                    for kt in range(OT):
                        nc.tensor.matmul(
                            py[:],
                            aoT_sb[:, kt, st * 128:(st + 1) * 128],
                            wo_sb[:, kt, l * 512:(l + 1) * 512],
                            start=(kt == 0), stop=(not with_bo and kt == OT - 1),
                        )
                    if with_bo:
                        nc.tensor.matmul(py[:], ones_t[0:1, 0:128],
                                         bor_sb[0:1, l * 512:(l + 1) * 512],
                                         start=False, stop=True)
                    nc.vector.tensor_copy(yt[:, l * 512:(l + 1) * 512], py[:])
                nc.sync.dma_start(out=y_d[st * 128:(st + 1) * 128, :], in_=yt[:])

            def flush_normalize():
                off2, p2, jb2, rb2, row = pending.pop(0)
                bt = pspool.tile([128, 512], f32, tag="ps", name="bt")
                nc.tensor.matmul(
                    bt[off2:off2 + 64, :],
                    ones_t[row:row + 1, 0:64],
                    rb2[row:row + 1, :],
                    start=True, stop=True,
                )
                nc.vector.tensor_mul(
                    aoT_sb[off2:off2 + 64, p2, jb2],
                    aoT_sb[off2:off2 + 64, p2, jb2],
                    bt[off2:off2 + 64, :])

            oproj_done = set()
            for p in range(OT):
                for c in range(4):
                    jb = slice(c * 512, (c + 1) * 512)
                    while len(pending) > 2:
                        flush_normalize()
                    otA = popool.tile([65, 512], f32, tag="po", name="otA")
                    otB = popool.tile([65, 512], f32, tag="po", name="otB")
                    for i in range(KT):
                        # combined scores psum: head A in cols 0-511 (rows
                        # 0-63 of the PE array), head B in cols 512-1023
                        # (rows 64-127) -- consecutive matmuls alternate PE
                        # row halves so the array overlaps them.
                        stt = pspool.tile([128, 1024], f32, tag="ps",
                                          name="stt")
                        for off in (0, 64):
                            nc.tensor.matmul(
                                stt[:, off * 8:off * 8 + 512],
                                kT_sb[off:off + 64, p, i * 128:(i + 1) * 128],
                                qT_sb[off:off + 64, p, jb],
                                start=True, stop=True,
                            )
                        et = wpool.tile([128, 1024], bf16, tag="exp", bufs=4,
                                        name="et")
                        nc.scalar.activation(et[:], stt[:], Exp, scale=0.125)
                        for ot, hh in ((otA, 0), (otB, 1)):
                            nc.tensor.matmul(
                                ot[:],
                                v_sb[:, i, 2 * p + hh, :],
                                et[:, hh * 512:(hh + 1) * 512],
                                start=(i == 0), stop=(i == KT - 1),
                            )
                    # Drain attn rows (cross-partition for the odd head) first
                    # so the PSUM tiles free fast, then gather the denominator
                    # rows to partitions 64 (A) / 32 (B) and batch-reciprocal.
                    nc.vector.tensor_copy(aoT_sb[0:64, p, jb], otA[0:64, :])
                    nc.vector.tensor_copy(aoT_sb[64:128, p, jb], otB[0:64, :])
                    den = wpool.tile([97, 512], f32, tag="den", bufs=2,
                                     name="den")
                    nc.vector.memset(den[64:97, :], 1.0)
                    nc.vector.tensor_copy(den[64:65, :], otA[64:65, :])
                    nc.vector.tensor_copy(den[96:97, :], otB[64:65, :])
                    rf = wpool.tile([97, 512], f32, tag="rf", name="rf")
                    rb = wpool.tile([65, 512], bf16, tag="rb", bufs=3,
                                    name="rb")
                    nc.vector.reciprocal(rf[64:97, :], den[64:97, :])
                    nc.vector.tensor_copy(rb[64:65, :], rf[64:65, :])
                    nc.vector.tensor_copy(rb[32:33, :], rf[96:97, :])
                    pending.append((0, p, jb, rb, 64))
                    pending.append((64, p, jb, rb, 32))
            while pending:
                flush_normalize()

            # ---- remaining output projection ----
            for st in range(ST):
                if st not in oproj_done:
                    emit_oproj_group(st)

    nc.compile()
    return nc


def get_nc(with_bv=True, with_bo=True):
    key = (with_bv, with_bo)
    if key not in _CACHED_NC:
        _CACHED_NC[key] = _build_nc(*key)
    return _CACHED_NC[key]


def make_in_maps(x, Wq, bq, Wk, bk, Wv, bv, Wo, bo):
    x = np.asarray(x, dtype=np.float32)
    in_maps = []
    for c in range(N_CORES):
        b, hg = c // 2, c % 2
        sl = slice(hg * HG_D, (hg + 1) * HG_D)
        in_maps.append({
            "xt": np.ascontiguousarray(np.asarray(x[b]).T).astype(BF16),
            "wq": np.ascontiguousarray(np.asarray(Wq)[:, sl]).astype(BF16),
            "wk": np.ascontiguousarray(np.asarray(Wk)[:, sl]).astype(BF16),
            "wv": np.ascontiguousarray(np.asarray(Wv)[:, sl]).astype(BF16),
            "wo": np.ascontiguousarray(np.asarray(Wo)[sl, :]).astype(BF16),
            "bqt": np.ascontiguousarray(
                np.asarray(bq, np.float32)[sl].reshape(OT, 128).T),
            "bkt": np.ascontiguousarray(
                np.asarray(bk, np.float32)[sl].reshape(OT, 128).T),
            "bvr": np.asarray(bv, np.float32)[sl].reshape(1, HG_D).astype(BF16),
            "bor": (np.asarray(bo, np.float32) if hg == 0
                    else np.zeros(D, np.float32)).reshape(1, D).astype(BF16),
        })
    return in_maps


def run_cores(in_maps, trace=False, with_bv=True, with_bo=True):
    try:
        import ntff_shim
        ntff_shim.install()
    except Exception:
        pass
    from concourse.bass_utils import run_bass_kernel_spmd

    nc = get_nc(with_bv, with_bo)
    return run_bass_kernel_spmd(nc, in_maps, list(range(N_CORES)), trace=trace)


def combine(results):
    y = np.empty((4, S, D), np.float32)
    for b in range(4):
        y[b] = results[2 * b]["y"] + results[2 * b + 1]["y"]
    return y


def kernel(x, Wq, bq, Wk, bk, Wv, bv, Wo, bo):
    in_maps = make_in_maps(x, Wq, bq, Wk, bk, Wv, bv, Wo, bo)
    with_bv = bool(np.any(np.asarray(bv)))
    with_bo = bool(np.any(np.asarray(bo)))
    res = run_cores(in_maps, trace=False, with_bv=with_bv, with_bo=with_bo)
    return combine(res.results)


# revision 34
# speedup vs baseline: 1.0997x; 1.0026x over previous
"""Multi-head attention (B=4, S=2048, D=1024, H=16, Hd=64) on 8 trn2 cores.

Sharding: core c = (batch b = c // 2, head-group hg = c % 2). Each core
computes attention for 8 heads of one batch and the corresponding slice of
the output projection; host sums the two partial outputs per batch.

Per-core layout (all matmuls bf16 with fp32 PSUM accumulation):
  xt   = x[b].T                    [D=1024, S=2048]  (lhsT/rhs K-major)
  qT/kT = (Wslice.T @ .. )         [512, 2048]  d-major, 4 pair-tiles of 128
  v    = x @ Wv_slice              [2048, 512]  s-major
  per head: scoresT[k,q] tile = kT.T-block @ qT  -> exp (ScalarE, scale=1/8)
            outT[d,q] += v-block.T @ expT ; denom[q] += ones.T @ expT
  normalize: outT *= broadcast(1/denom) via K=1 ones-matmul
  y = outT.T-blocks @ Wo_slice + bo   [2048, 1024] fp32 partial
"""

import numpy as np
import ml_dtypes

S = 2048
D = 1024
HG_D = 512          # head dims per core (8 heads x 64)
NH = 8              # heads per core
KT = S // 128       # 16 k-tiles
DT = D // 128       # 8 contraction tiles for QKV
ST = S // 128       # 16 s-tiles
OT = HG_D // 128    # 4 contraction tiles for O-proj / pair tiles
N_CORES = 8

BF16 = ml_dtypes.bfloat16

_CACHED_NC = {}


def _build_nc(with_bv=True, with_bo=True):
    import concourse.bass as bass  # noqa: F401
    import concourse.mybir as mybir
    import concourse.tile as tile
    from concourse import bacc

    f32 = mybir.dt.float32
    bf16 = mybir.dt.bfloat16
    fp16 = mybir.dt.float16
    Exp = mybir.ActivationFunctionType.Exp

    nc = bacc.Bacc("TRN2", target_bir_lowering=False, debug=False,
                   num_devices=N_CORES)

    xt_d = nc.dram_tensor("xt", [D, S], bf16, kind="ExternalInput")
    wq_d = nc.dram_tensor("wq", [D, HG_D], bf16, kind="ExternalInput")
    wk_d = nc.dram_tensor("wk", [D, HG_D], bf16, kind="ExternalInput")
    wv_d = nc.dram_tensor("wv", [D, HG_D], bf16, kind="ExternalInput")
    wo_d = nc.dram_tensor("wo", [HG_D, D], bf16, kind="ExternalInput")
    bq_d = nc.dram_tensor("bqt", [128, OT], f32, kind="ExternalInput")
    bk_d = nc.dram_tensor("bkt", [128, OT], f32, kind="ExternalInput")
    bv_d = nc.dram_tensor("bvr", [1, HG_D], bf16, kind="ExternalInput")
    bo_d = nc.dram_tensor("bor", [1, D], bf16, kind="ExternalInput")
    y_d = nc.dram_tensor("y", [S, D], f32, kind="ExternalOutput")

    with tile.TileContext(nc) as tc:
        with (
            tc.tile_pool(name="cpool", bufs=1) as cpool,
            tc.tile_pool(name="wpool", bufs=2) as wpool,
            tc.tile_pool(name="pspool", bufs=3, space="PSUM") as pspool,
            tc.tile_pool(name="popool", bufs=2, space="PSUM") as popool,
        ):
            # ---- persistent SBUF tiles ----
            xt_sb = cpool.tile([128, DT, S], bf16, name="xt_sb")
            wq_sb = cpool.tile([128, DT, HG_D], bf16, name="wq_sb")
            wk_sb = cpool.tile([128, DT, HG_D], bf16, name="wk_sb")
            wv_sb = cpool.tile([128, DT, HG_D], bf16, name="wv_sb")
            wo_sb = cpool.tile([128, OT, D], bf16, name="wo_sb")
            bq_sb = cpool.tile([128, OT], f32, name="bq_sb")
            bk_sb = cpool.tile([128, OT], f32, name="bk_sb")
            bvr_sb = cpool.tile([1, HG_D], bf16, name="bvr_sb")
            bor_sb = cpool.tile([1, D], bf16, name="bor_sb")
            ones_t = cpool.tile([128, 128], bf16, name="ones_t")
            qT_sb = cpool.tile([128, OT, S], bf16, name="qT_sb")
            kT_sb = cpool.tile([128, OT, S], bf16, name="kT_sb")
            # v with a trailing ones column per head: attnv lhsT [128, 65]
            # whose 65th output row accumulates the softmax denominator.
            v_sb = cpool.tile([128, ST, NH, 65], bf16, name="v_sb")
            aoT_sb = cpool.tile([128, OT, S], bf16, name="aoT_sb")

            # ---- loads ----
            for k in range(DT):
                nc.sync.dma_start(out=xt_sb[:, k, :], in_=xt_d[k * 128:(k + 1) * 128, :])
                nc.sync.dma_start(out=wq_sb[:, k, :], in_=wq_d[k * 128:(k + 1) * 128, :])
                nc.sync.dma_start(out=wk_sb[:, k, :], in_=wk_d[k * 128:(k + 1) * 128, :])
                nc.sync.dma_start(out=wv_sb[:, k, :], in_=wv_d[k * 128:(k + 1) * 128, :])
            for k in range(OT):
                nc.sync.dma_start(out=wo_sb[:, k, :], in_=wo_d[k * 128:(k + 1) * 128, :])
            nc.sync.dma_start(out=bq_sb[:], in_=bq_d[:])
            nc.sync.dma_start(out=bk_sb[:], in_=bk_d[:])
            nc.sync.dma_start(out=bvr_sb[:], in_=bv_d[:])
            nc.sync.dma_start(out=bor_sb[:], in_=bo_d[:])
            nc.gpsimd.memset(ones_t[:], 1.0)

            # ---- projections: qT/kT [d-major], bias via per-partition scalar ----
            def emit_qk_group(w_sb, b_sb, out_sb, p, jc):
                pq = pspool.tile([128, 512], f32, tag="ps", name="pq")
                for k in range(DT):
                    nc.tensor.matmul(
                        pq[:],
                        w_sb[:, k, p * 128:(p + 1) * 128],
                        xt_sb[:, k, jc * 512:(jc + 1) * 512],
                        start=(k == 0), stop=(k == DT - 1),
                    )
                nc.vector.tensor_scalar_add(
                    out_sb[:, p, jc * 512:(jc + 1) * 512], pq[:], b_sb[:, p:p + 1])

            for w_sb, b_sb, out_sb in ((wq_sb, bq_sb, qT_sb), (wk_sb, bk_sb, kT_sb)):
                for p in range(OT):
                    for jc in range(4):
                        emit_qk_group(w_sb, b_sb, out_sb, p, jc)

            # ---- v projection [s-major], bias via K=1 ones matmul ----
            nc.vector.memset(v_sb[:], 1.0)
            for st in range(ST):
                pv = pspool.tile([128, 512], f32, tag="ps", name="pv")
                for k in range(DT):
                    nc.tensor.matmul(
                        pv[:],
                        xt_sb[:, k, st * 128:(st + 1) * 128],
                        wv_sb[:, k, :],
                        start=(k == 0), stop=(not with_bv and k == DT - 1),
                    )
                if with_bv:
                    nc.tensor.matmul(pv[:], ones_t[0:1, 0:128], bvr_sb[0:1, :],
                                     start=False, stop=True)
                nc.vector.tensor_copy(
                    v_sb[:, st, :, 0:64],
                    pv.rearrange("p (h c) -> p h c", c=64))

            # ---- attention, one head PAIR per chunk ----
            # Both heads of a pair are processed together so consecutive
            # scores matmuls alternate PE row halves (rows 0-63 / 64-127),
            # which lets the hardware overlap them (~2x scores throughput).
            # Normalization (broadcast of 1/denom + scale) is deferred by one
            # chunk so the PE never waits on the slow DVE reciprocal.
            pending = []

            def emit_oproj_group(st):
                yt = wpool.tile([128, D], f32, tag="y", name="yt")
                for l in range(2):
                    py = pspool.tile([128, 512], f32, tag="ps", name="py")
                    for kt in range(OT):
                        nc.tensor.matmul(
                            py[:],
                            aoT_sb[:, kt, st * 128:(st + 1) * 128],
                            wo_sb[:, kt, l * 512:(l + 1) * 512],
                            start=(kt == 0), stop=(not with_bo and kt == OT - 1),
                        )
                    if with_bo:
                        nc.tensor.matmul(py[:], ones_t[0:1, 0:128],
                                         bor_sb[0:1, l * 512:(l + 1) * 512],
                                         start=False, stop=True)
                    nc.vector.tensor_copy(yt[:, l * 512:(l + 1) * 512], py[:])
                nc.sync.dma_start(out=y_d[st * 128:(st + 1) * 128, :], in_=yt[:])

            def flush_normalize():
                off2, p2, jb2, rb2, row = pending.pop(0)
                bt = pspool.tile([128, 512], f32, tag="ps", name="bt")
                nc.tensor.matmul(
                    bt[off2:off2 + 64, :],
                    ones_t[row:row + 1, 0:64],
                    rb2[row:row + 1, :],
                    start=True, stop=True,
                )
                nc.vector.tensor_mul(
                    aoT_sb[off2:off2 + 64, p2, jb2],
                    aoT_sb[off2:off2 + 64, p2, jb2],
                    bt[off2:off2 + 64, :])

            oproj_done = set()
            for p in range(OT):
                for c in range(4):
                    jb = slice(c * 512, (c + 1) * 512)
                    while len(pending) > 2:
                        flush_normalize()
                    otA = popool.tile([65, 512], f32, tag="po", name="otA")
                    otB = popool.tile([65, 512], f32, tag="po", name="otB")
                    for i in range(KT):
                        # combined scores psum: head A in cols 0-511 (rows
                        # 0-63 of the PE array), head B in cols 512-1023
                        # (rows 64-127) -- consecutive matmuls alternate PE
                        # row halves so the array overlaps them.
                        stt = pspool.tile([128, 1024], f32, tag="ps",
                                          name="stt")
                        for off in (0, 64):
                            nc.tensor.matmul(
                                stt[:, off * 8:off * 8 + 512],
                                kT_sb[off:off + 64, p, i * 128:(i + 1) * 128],
                                qT_sb[off:off + 64, p, jb],
                                start=True, stop=True,
                            )
                        et = wpool.tile([128, 1024], bf16, tag="exp", bufs=4,
                                        name="et")
                        nc.scalar.activation(et[:], stt[:], Exp, scale=0.125)
                        for ot, hh in ((otA, 0), (otB, 1)):
                            nc.tensor.matmul(
                                ot[:],
                                v_sb[:, i, 2 * p + hh, :],
                                et[:, hh * 512:(hh + 1) * 512],
                                start=(i == 0), stop=(i == KT - 1),
                            )
                    # Drain attn rows (cross-partition for the odd head) first
                    # so the PSUM tiles free fast, then gather the denominator
                    # rows to partitions 64 (A) / 32 (B) and batch-reciprocal.
                    nc.vector.tensor_copy(aoT_sb[0:64, p, jb], otA[0:64, :])
                    nc.vector.tensor_copy(aoT_sb[64:128, p, jb], otB[0:64, :])
                    den = wpool.tile([97, 512], f32, tag="den", bufs=2,
                                     name="den")
                    nc.vector.memset(den[64:97, :], 1.0)
                    nc.vector.tensor_copy(den[64:65, :], otA[64:65, :])
                    nc.vector.tensor_copy(den[96:97, :], otB[64:65, :])
                    rf = wpool.tile([97, 512], f32, tag="rf", name="rf")
                    rb = wpool.tile([65, 512], bf16, tag="rb", bufs=3,
                                    name="rb")
                    nc.vector.reciprocal(rf[64:97, :], den[64:97, :])
                    nc.vector.tensor_copy(rb[64:65, :], rf[64:65, :])
                    nc.vector.tensor_copy(rb[32:33, :], rf[96:97, :])
                    pending.append((0, p, jb, rb, 64))
                    pending.append((64, p, jb, rb, 32))
            while pending:
                flush_normalize()

            # ---- remaining output projection ----
            for st in range(ST):
                if st not in oproj_done:
                    emit_oproj_group(st)

    nc.compile()
    return nc


def get_nc(with_bv=True, with_bo=True):
    key = (with_bv, with_bo)
    if key not in _CACHED_NC:
        _CACHED_NC[key] = _build_nc(*key)
    return _CACHED_NC[key]


def make_in_maps(x, Wq, bq, Wk, bk, Wv, bv, Wo, bo):
    x = np.asarray(x, dtype=np.float32)
    in_maps = []
    for c in range(N_CORES):
        b, hg = c // 2, c % 2
        sl = slice(hg * HG_D, (hg + 1) * HG_D)
        in_maps.append({
            "xt": np.ascontiguousarray(np.asarray(x[b]).T).astype(BF16),
            "wq": np.ascontiguousarray(np.asarray(Wq)[:, sl]).astype(BF16),
            "wk": np.ascontiguousarray(np.asarray(Wk)[:, sl]).astype(BF16),
            "wv": np.ascontiguousarray(np.asarray(Wv)[:, sl]).astype(BF16),
            "wo": np.ascontiguousarray(np.asarray(Wo)[sl, :]).astype(BF16),
            "bqt": np.ascontiguousarray(
                np.asarray(bq, np.float32)[sl].reshape(OT, 128).T),
            "bkt": np.ascontiguousarray(
                np.asarray(bk, np.float32)[sl].reshape(OT, 128).T),
            "bvr": np.asarray(bv, np.float32)[sl].reshape(1, HG_D).astype(BF16),
            "bor": (np.asarray(bo, np.float32) if hg == 0
                    else np.zeros(D, np.float32)).reshape(1, D).astype(BF16),
        })
    return in_maps


def run_cores(in_maps, trace=False, with_bv=True, with_bo=True):
    try:
        import ntff_shim
        ntff_shim.install()
    except Exception:
        pass
    from concourse.bass_utils import run_bass_kernel_spmd

    nc = get_nc(with_bv, with_bo)
    return run_bass_kernel_spmd(nc, in_maps, list(range(N_CORES)), trace=trace)


def combine(results):
    y = np.empty((4, S, D), np.float32)
    for b in range(4):
        y[b] = results[2 * b]["y"] + results[2 * b + 1]["y"]
    return y


def kernel(x, Wq, bq, Wk, bk, Wv, bv, Wo, bo):
    in_maps = make_in_maps(x, Wq, bq, Wk, bk, Wv, bv, Wo, bo)
    with_bv = bool(np.any(np.asarray(bv)))
    with_bo = bool(np.any(np.asarray(bo)))
    res = run_cores(in_maps, trace=False, with_bv=with_bv, with_bo=with_bo)
    return combine(res.results)


# revision 35
# speedup vs baseline: 1.1220x; 1.0203x over previous
"""Multi-head attention (B=4, S=2048, D=1024, H=16, Hd=64) on 8 trn2 cores.

Sharding: core c = (batch b = c // 2, head-group hg = c % 2). Each core
computes attention for 8 heads of one batch and the corresponding slice of
the output projection; host sums the two partial outputs per batch.

Per-core layout (all matmuls bf16 with fp32 PSUM accumulation):
  xt   = x[b].T                    [D=1024, S=2048]  (lhsT/rhs K-major)
  qT/kT = (Wslice.T @ .. )         [512, 2048]  d-major, 4 pair-tiles of 128
  v    = x @ Wv_slice              [2048, 512]  s-major
  per head: scoresT[k,q] tile = kT.T-block @ qT  -> exp (ScalarE, scale=1/8)
            outT[d,q] += v-block.T @ expT ; denom[q] += ones.T @ expT
  normalize: outT *= broadcast(1/denom) via K=1 ones-matmul
  y = outT.T-blocks @ Wo_slice + bo   [2048, 1024] fp32 partial
"""

import numpy as np
import ml_dtypes

S = 2048
D = 1024
HG_D = 512          # head dims per core (8 heads x 64)
NH = 8              # heads per core
KT = S // 128       # 16 k-tiles
DT = D // 128       # 8 contraction tiles for QKV
ST = S // 128       # 16 s-tiles
OT = HG_D // 128    # 4 contraction tiles for O-proj / pair tiles
N_CORES = 8

BF16 = ml_dtypes.bfloat16

_CACHED_NC = {}


def _build_nc(with_bv=True, with_bo=True):
    import concourse.bass as bass  # noqa: F401
    import concourse.mybir as mybir
    import concourse.tile as tile
    from concourse import bacc

    f32 = mybir.dt.float32
    bf16 = mybir.dt.bfloat16
    fp16 = mybir.dt.float16
    Exp = mybir.ActivationFunctionType.Exp

    nc = bacc.Bacc("TRN2", target_bir_lowering=False, debug=False,
                   num_devices=N_CORES)

    xt_d = nc.dram_tensor("xt", [D, S], bf16, kind="ExternalInput")
    wq_d = nc.dram_tensor("wq", [D, HG_D], bf16, kind="ExternalInput")
    wk_d = nc.dram_tensor("wk", [D, HG_D], bf16, kind="ExternalInput")
    wv_d = nc.dram_tensor("wv", [D, HG_D], bf16, kind="ExternalInput")
    wo_d = nc.dram_tensor("wo", [HG_D, D], bf16, kind="ExternalInput")
    bq_d = nc.dram_tensor("bqt", [128, OT], f32, kind="ExternalInput")
    bk_d = nc.dram_tensor("bkt", [128, OT], f32, kind="ExternalInput")
    bv_d = nc.dram_tensor("bvr", [1, HG_D], bf16, kind="ExternalInput")
    bo_d = nc.dram_tensor("bor", [1, D], bf16, kind="ExternalInput")
    y_d = nc.dram_tensor("y", [S, D], f32, kind="ExternalOutput")

    with tile.TileContext(nc) as tc:
        with (
            tc.tile_pool(name="cpool", bufs=1) as cpool,
            tc.tile_pool(name="wpool", bufs=2) as wpool,
            tc.tile_pool(name="pspool", bufs=3, space="PSUM") as pspool,
            tc.tile_pool(name="popool", bufs=2, space="PSUM") as popool,
        ):
            # ---- persistent SBUF tiles ----
            xt_sb = cpool.tile([128, DT, S], bf16, name="xt_sb")
            wq_sb = cpool.tile([128, DT, HG_D], bf16, name="wq_sb")
            wk_sb = cpool.tile([128, DT, HG_D], bf16, name="wk_sb")
            wv_sb = cpool.tile([128, DT, HG_D], bf16, name="wv_sb")
            wo_sb = cpool.tile([128, OT, D], bf16, name="wo_sb")
            bq_sb = cpool.tile([128, OT], f32, name="bq_sb")
            bk_sb = cpool.tile([128, OT], f32, name="bk_sb")
            bvr_sb = cpool.tile([1, HG_D], bf16, name="bvr_sb")
            bor_sb = cpool.tile([1, D], bf16, name="bor_sb")
            ones_t = cpool.tile([128, 128], bf16, name="ones_t")
            qT_sb = cpool.tile([128, OT, S], bf16, name="qT_sb")
            kT_sb = cpool.tile([128, OT, S], bf16, name="kT_sb")
            # v with a trailing ones column per head: attnv lhsT [128, 65]
            # whose 65th output row accumulates the softmax denominator.
            v_sb = cpool.tile([128, ST, NH, 65], bf16, name="v_sb")
            aoT_sb = cpool.tile([128, OT, S], bf16, name="aoT_sb")

            # ---- loads ----
            for k in range(DT):
                nc.sync.dma_start(out=xt_sb[:, k, :], in_=xt_d[k * 128:(k + 1) * 128, :])
                nc.sync.dma_start(out=wq_sb[:, k, :], in_=wq_d[k * 128:(k + 1) * 128, :])
                nc.sync.dma_start(out=wk_sb[:, k, :], in_=wk_d[k * 128:(k + 1) * 128, :])
                nc.sync.dma_start(out=wv_sb[:, k, :], in_=wv_d[k * 128:(k + 1) * 128, :])
            for k in range(OT):
                nc.sync.dma_start(out=wo_sb[:, k, :], in_=wo_d[k * 128:(k + 1) * 128, :])
            nc.sync.dma_start(out=bq_sb[:], in_=bq_d[:])
            nc.sync.dma_start(out=bk_sb[:], in_=bk_d[:])
            nc.sync.dma_start(out=bvr_sb[:], in_=bv_d[:])
            nc.sync.dma_start(out=bor_sb[:], in_=bo_d[:])
            nc.gpsimd.memset(ones_t[:], 1.0)

            # ---- projections: qT/kT [d-major], bias via per-partition scalar ----
            def emit_qk_group(w_sb, b_sb, out_sb, p, jc):
                pq = pspool.tile([128, 512], f32, tag="ps", name="pq")
                for k in range(DT):
                    nc.tensor.matmul(
                        pq[:],
                        w_sb[:, k, p * 128:(p + 1) * 128],
                        xt_sb[:, k, jc * 512:(jc + 1) * 512],
                        start=(k == 0), stop=(k == DT - 1),
                    )
                nc.vector.tensor_scalar_add(
                    out_sb[:, p, jc * 512:(jc + 1) * 512], pq[:], b_sb[:, p:p + 1])

            for w_sb, b_sb, out_sb in ((wq_sb, bq_sb, qT_sb), (wk_sb, bk_sb, kT_sb)):
                for p in range(OT):
                    for jc in range(4):
                        emit_qk_group(w_sb, b_sb, out_sb, p, jc)

            # ---- v projection [s-major], bias via K=1 ones matmul ----
            nc.vector.memset(v_sb[:], 1.0)
            for st in range(ST):
                pv = pspool.tile([128, 512], f32, tag="ps", name="pv")
                for k in range(DT):
                    nc.tensor.matmul(
                        pv[:],
                        xt_sb[:, k, st * 128:(st + 1) * 128],
                        wv_sb[:, k, :],
                        start=(k == 0), stop=(not with_bv and k == DT - 1),
                    )
                if with_bv:
                    nc.tensor.matmul(pv[:], ones_t[0:1, 0:128], bvr_sb[0:1, :],
                                     start=False, stop=True)
                nc.vector.tensor_copy(
                    v_sb[:, st, :, 0:64],
                    pv.rearrange("p (h c) -> p h c", c=64))

            # ---- attention, one head PAIR per chunk ----
            # Both heads of a pair are processed together so consecutive
            # scores matmuls alternate PE row halves (rows 0-63 / 64-127),
            # which lets the hardware overlap them (~2x scores throughput).
            # Normalization (broadcast of 1/denom + scale) is deferred by one
            # chunk so the PE never waits on the slow DVE reciprocal.
            pending = []

            def emit_oproj_group(st):
                yt = wpool.tile([128, D], f32, tag="y", bufs=3, name="yt")
                for l in range(2):
                    py = pspool.tile([128, 512], f32, tag="ps", name="py")
                    for kt in range(OT):
                        nc.tensor.matmul(
                            py[:],
                            aoT_sb[:, kt, st * 128:(st + 1) * 128],
                            wo_sb[:, kt, l * 512:(l + 1) * 512],
                            start=(kt == 0), stop=(not with_bo and kt == OT - 1),
                        )
                    if with_bo:
                        nc.tensor.matmul(py[:], ones_t[0:1, 0:128],
                                         bor_sb[0:1, l * 512:(l + 1) * 512],
                                         start=False, stop=True)
                    nc.vector.tensor_copy(yt[:, l * 512:(l + 1) * 512], py[:])
                nc.sync.dma_start(out=y_d[st * 128:(st + 1) * 128, :], in_=yt[:])

            def flush_normalize():
                off2, p2, jb2, rb2, row = pending.pop(0)
                bt = pspool.tile([128, 512], f32, tag="ps", name="bt")
                nc.tensor.matmul(
                    bt[off2:off2 + 64, :],
                    ones_t[row:row + 1, 0:64],
                    rb2[row:row + 1, :],
                    start=True, stop=True,
                )
                nc.vector.tensor_mul(
                    aoT_sb[off2:off2 + 64, p2, jb2],
                    aoT_sb[off2:off2 + 64, p2, jb2],
                    bt[off2:off2 + 64, :])

            oproj_done = set()
            for p in range(OT):
                for c in range(4):
                    jb = slice(c * 512, (c + 1) * 512)
                    while len(pending) > 2:
                        flush_normalize()
                    otA = popool.tile([65, 512], f32, tag="po", name="otA")
                    otB = popool.tile([65, 512], f32, tag="po", name="otB")
                    for i in range(KT):
                        # combined scores psum: head A in cols 0-511 (rows
                        # 0-63 of the PE array), head B in cols 512-1023
                        # (rows 64-127) -- consecutive matmuls alternate PE
                        # row halves so the array overlaps them.
                        stt = pspool.tile([128, 1024], f32, tag="ps",
                                          name="stt")
                        for off in (0, 64):
                            nc.tensor.matmul(
                                stt[:, off * 8:off * 8 + 512],
                                kT_sb[off:off + 64, p, i * 128:(i + 1) * 128],
                                qT_sb[off:off + 64, p, jb],
                                start=True, stop=True,
                            )
                        et = wpool.tile([128, 1024], bf16, tag="exp", bufs=4,
                                        name="et")
                        nc.scalar.activation(et[:], stt[:], Exp, scale=0.125)
                        for ot, hh in ((otA, 0), (otB, 1)):
                            nc.tensor.matmul(
                                ot[:],
                                v_sb[:, i, 2 * p + hh, :],
                                et[:, hh * 512:(hh + 1) * 512],
                                start=(i == 0), stop=(i == KT - 1),
                            )
                    # Drain attn rows (cross-partition for the odd head) first
                    # so the PSUM tiles free fast, then gather the denominator
                    # rows to partitions 64 (A) / 32 (B) and batch-reciprocal.
                    nc.vector.tensor_copy(aoT_sb[0:64, p, jb], otA[0:64, :])
                    nc.vector.tensor_copy(aoT_sb[64:128, p, jb], otB[0:64, :])
                    den = wpool.tile([97, 512], f32, tag="den", bufs=2,
                                     name="den")
                    nc.vector.memset(den[64:97, :], 1.0)
                    nc.vector.tensor_copy(den[64:65, :], otA[64:65, :])
                    nc.vector.tensor_copy(den[96:97, :], otB[64:65, :])
                    rf = wpool.tile([97, 512], f32, tag="rf", name="rf")
                    rb = wpool.tile([65, 512], bf16, tag="rb", bufs=3,
                                    name="rb")
                    nc.vector.reciprocal(rf[64:97, :], den[64:97, :])
                    nc.vector.tensor_copy(rb[64:65, :], rf[64:65, :])
                    nc.vector.tensor_copy(rb[32:33, :], rf[96:97, :])
                    pending.append((0, p, jb, rb, 64))
                    pending.append((64, p, jb, rb, 32))
            while pending:
                flush_normalize()

            # ---- remaining output projection ----
            for st in range(ST):
                if st not in oproj_done:
                    emit_oproj_group(st)

    nc.compile()
    return nc


def get_nc(with_bv=True, with_bo=True):
    key = (with_bv, with_bo)
    if key not in _CACHED_NC:
        _CACHED_NC[key] = _build_nc(*key)
    return _CACHED_NC[key]


def make_in_maps(x, Wq, bq, Wk, bk, Wv, bv, Wo, bo):
    x = np.asarray(x, dtype=np.float32)
    in_maps = []
    for c in range(N_CORES):
        b, hg = c // 2, c % 2
        sl = slice(hg * HG_D, (hg + 1) * HG_D)
        in_maps.append({
            "xt": np.ascontiguousarray(np.asarray(x[b]).T).astype(BF16),
            "wq": np.ascontiguousarray(np.asarray(Wq)[:, sl]).astype(BF16),
            "wk": np.ascontiguousarray(np.asarray(Wk)[:, sl]).astype(BF16),
            "wv": np.ascontiguousarray(np.asarray(Wv)[:, sl]).astype(BF16),
            "wo": np.ascontiguousarray(np.asarray(Wo)[sl, :]).astype(BF16),
            "bqt": np.ascontiguousarray(
                np.asarray(bq, np.float32)[sl].reshape(OT, 128).T),
            "bkt": np.ascontiguousarray(
                np.asarray(bk, np.float32)[sl].reshape(OT, 128).T),
            "bvr": np.asarray(bv, np.float32)[sl].reshape(1, HG_D).astype(BF16),
            "bor": (np.asarray(bo, np.float32) if hg == 0
                    else np.zeros(D, np.float32)).reshape(1, D).astype(BF16),
        })
    return in_maps


def run_cores(in_maps, trace=False, with_bv=True, with_bo=True):
    try:
        import ntff_shim
        ntff_shim.install()
    except Exception:
        pass
    from concourse.bass_utils import run_bass_kernel_spmd

    nc = get_nc(with_bv, with_bo)
    return run_bass_kernel_spmd(nc, in_maps, list(range(N_CORES)), trace=trace)


def combine(results):
    y = np.empty((4, S, D), np.float32)
    for b in range(4):
        y[b] = results[2 * b]["y"] + results[2 * b + 1]["y"]
    return y


def kernel(x, Wq, bq, Wk, bk, Wv, bv, Wo, bo):
    in_maps = make_in_maps(x, Wq, bq, Wk, bk, Wv, bv, Wo, bo)
    with_bv = bool(np.any(np.asarray(bv)))
    with_bo = bool(np.any(np.asarray(bo)))
    res = run_cores(in_maps, trace=False, with_bv=with_bv, with_bo=with_bo)
    return combine(res.results)
